# revision 6
# baseline (speedup 1.0000x reference)
"""Trainium2 Bass kernel for nn_Block_50113678410401 (dense transformer block).

Strategy: data-parallel over the batch axis (B=8 -> 8 NeuronCores, one batch
element per core). All on-chip activations live in "layout A": feature axis on
SBUF partitions, token axis (T) on the free dimension, so no on-chip
transposes are needed (host pre-transposes x and post-transposes the output).

Per core:
  LN1 (stats via ones-matmul over partitions), per-head causal attention
  (no-max-sub exp softmax, denominator via ones-matmul, normalization via
  K=1 broadcast matmul), output projection + residual, BatchNorm over (B,C)
  with a cross-core AllReduce of (sum, sumsq) per T channel, LN2, FFN
  (C -> 4C -> relu -> C), residual, second BatchNorm (second AllReduce).

All big matmuls run in bf16 with fp32 PSUM accumulation; statistics,
softmax, residuals and normalizations are fp32.

LayerNorm/projection affine parameters are folded into the weights on the
host: wq' = diag(ln1_g) wq / sqrt(D) (q also carries 1/sqrt(D)), k-side bias
drops out of softmax by shift invariance, v-side bias is folded into the
output-projection bias, ln2 affine is folded into w1/b1.
"""

import numpy as np
import ml_dtypes

B, T, C, H, D = 8, 1024, 1536, 12, 128
F = 4 * C            # 6144
P = 128
CT = C // P          # 12 c-tiles
FT = F // P          # 48 f-tiles
ST = T // P          # 8 s-tiles
CH = 512             # matmul free-dim chunk
NCH = T // CH        # 2 chunks
EPS = 1e-5
NCORES = 8
NBC = B * C          # BatchNorm count over (B, C)

_PROG = None


def _build():
    import concourse.bass as bass
    import concourse.mybir as mybir
    import concourse.tile as tile
    from concourse import bacc
    from concourse.masks import make_upper_triangular

    fp32 = mybir.dt.float32
    bf16 = mybir.dt.bfloat16
    AF = mybir.ActivationFunctionType
    OP = mybir.AluOpType
    ts = bass.ts

    nc = bacc.Bacc("TRN2", target_bir_lowering=False, debug=False,
                   enable_asserts=True, num_devices=NCORES)

    # ---- DRAM I/O ----
    xT_d = nc.dram_tensor("xT", (C, T), fp32, kind="ExternalInput").ap()
    wq_d = nc.dram_tensor("wq", (C, C), bf16, kind="ExternalInput").ap()
    wk_d = nc.dram_tensor("wk", (C, C), bf16, kind="ExternalInput").ap()
    wv_d = nc.dram_tensor("wv", (C, C), bf16, kind="ExternalInput").ap()
    bq_d = nc.dram_tensor("bq", (P, H), fp32, kind="ExternalInput").ap()
    wo_d = nc.dram_tensor("wo", (C, C), bf16, kind="ExternalInput").ap()
    bo_d = nc.dram_tensor("bo", (P, CT), fp32, kind="ExternalInput").ap()
    w1_d = nc.dram_tensor("w1", (C, F), bf16, kind="ExternalInput").ap()
    b1_d = nc.dram_tensor("b1", (P, FT), fp32, kind="ExternalInput").ap()
    w2_d = nc.dram_tensor("w2", (F, C), bf16, kind="ExternalInput").ap()
    b2_d = nc.dram_tensor("b2", (P, CT), fp32, kind="ExternalInput").ap()
    bn1g_d = nc.dram_tensor("bn1g", (1, T), fp32, kind="ExternalInput").ap()
    bn1b_d = nc.dram_tensor("bn1b", (1, T), fp32, kind="ExternalInput").ap()
    bn2g_d = nc.dram_tensor("bn2g", (1, T), fp32, kind="ExternalInput").ap()
    bn2b_d = nc.dram_tensor("bn2b", (1, T), fp32, kind="ExternalInput").ap()
    yT_d = nc.dram_tensor("yT", (C, T), fp32, kind="ExternalOutput").ap()

    wq_r = wq_d.rearrange("(ko p) n -> p ko n", p=P)
    wk_r = wk_d.rearrange("(ko p) n -> p ko n", p=P)
    wo_r = wo_d.rearrange("(ho p) n -> p ho n", p=P)
    w1_r = w1_d.rearrange("(ko p) n -> p ko n", p=P)
    w2_r = w2_d.rearrange("(fo p) n -> p fo n", p=P)

    with tile.TileContext(nc) as tc:
        with tc.tile_pool(name="const", bufs=1) as cpool, \
             tc.tile_pool(name="scratch", bufs=1) as spool, \
             tc.tile_pool(name="u1p", bufs=1) as u1pool, \
             tc.tile_pool(name="ppw", bufs=4, space="PSUM") as ppw, \
             tc.tile_pool(name="pps", bufs=4, space="PSUM") as pps, \
             tc.tile_pool(name="dram", bufs=1, space="DRAM") as dpool:

            # ---- constants ----
            ones_bf = cpool.tile([P, 1], bf16, name="ones_bf")
            nc.vector.memset(ones_bf[:], 1.0)
            ones1f = cpool.tile([1, P], fp32, name="ones1f")
            nc.vector.memset(ones1f[:], 1.0)
            trimask = cpool.tile([P, P], bf16, name="trimask")
            make_upper_triangular(nc, trimask[:], val=1.0, diag=True)
            bq_sb = cpool.tile([P, H], fp32, name="bq_sb")
            nc.sync.dma_start(bq_sb[:], bq_d[:])
            bo_sb = cpool.tile([P, CT], fp32, name="bo_sb")
            nc.sync.dma_start(bo_sb[:], bo_d[:])
            b1_sb = cpool.tile([P, FT], fp32, name="b1_sb")
            nc.sync.dma_start(b1_sb[:], b1_d[:])
            b2_sb = cpool.tile([P, CT], fp32, name="b2_sb")
            nc.sync.dma_start(b2_sb[:], b2_d[:])

            # ---- helpers ----
            def stats_accum(src_tile, s1_ps, s2_ps, first, last):
                """Ones-matmul partial sums of src (fp32 (P,T)) and its
                square into psum rows (1, CH) x NCH."""
                for j in range(NCH):
                    sl = slice(j * CH, (j + 1) * CH)
                    cbf = spool.tile([P, CH], bf16, tag="cast_bf", bufs=3,
                                     name="cbf")
                    nc.vector.tensor_copy(cbf[:], src_tile[:, sl])
                    csq = spool.tile([P, CH], bf16, tag="cast_sq", bufs=3,
                                     name="csq")
                    nc.scalar.square(csq[:], src_tile[:, sl])
                    nc.tensor.matmul(s1_ps[j][:], ones_bf[:], cbf[:],
                                     start=first, stop=last)
                    nc.tensor.matmul(s2_ps[j][:], ones_bf[:], csq[:],
                                     start=first, stop=last)

            def rows_from_ps(pool, ps_list, name):
                row = pool.tile([1, T], fp32, name=name)
                for j in range(NCH):
                    nc.scalar.copy(row[:, j * CH:(j + 1) * CH], ps_list[j][:])
                return row

            def norm_rows(pool, s1row, s2row, count, name,
                          g_row=None, b_row=None):
                """scale = (1/sqrt(var+eps)) * g ; bias = -mean*scale + b."""
                m = pool.tile([1, T], fp32, name=f"{name}_m")
                nc.vector.tensor_scalar_mul(m[:], s1row, 1.0 / count)
                v = pool.tile([1, T], fp32, name=f"{name}_v")
                nc.vector.tensor_scalar_mul(v[:], s2row, 1.0 / count)
                msq = pool.tile([1, T], fp32, name=f"{name}_msq")
                nc.vector.tensor_mul(msq[:], m[:], m[:])
                nc.vector.tensor_sub(v[:], v[:], msq[:])
                nc.vector.tensor_scalar_add(v[:], v[:], EPS)
                nc.scalar.sqrt(v[:], v[:])
                scale = pool.tile([1, T], fp32, name=f"{name}_scale")
                nc.vector.reciprocal(scale[:], v[:])
                if g_row is not None:
                    nc.vector.tensor_mul(scale[:], scale[:], g_row[:])
                bias = pool.tile([1, T], fp32, name=f"{name}_bias")
                nc.vector.tensor_mul(bias[:], m[:], scale[:])
                nc.vector.tensor_scalar_mul(bias[:], bias[:], -1.0)
                if b_row is not None:
                    nc.vector.tensor_add(bias[:], bias[:], b_row[:])
                return scale, bias

            def broadcast_row(pool, row, name, tag="bc", bufs=2):
                """(1, T) fp32 -> (P, T) fp32 via K=1 matmul."""
                bc = pool.tile([P, T], fp32, tag=tag, bufs=bufs, name=name)
                for j in range(NCH):
                    sl = slice(j * CH, (j + 1) * CH)
                    ps = ppw.tile([P, CH], fp32, tag="w", name=f"{name}_ps")
                    nc.tensor.matmul(ps[:], ones1f[:], row[:, sl],
                                     start=True, stop=True)
                    nc.scalar.copy(bc[:, sl], ps[:])
                return bc

            def allreduce_rows(pool, s1_ps, s2_ps, name):
                """Assemble (sum, sumsq) rows, AllReduce-add across cores."""
                loc = pool.tile([1, 2 * T], fp32, name=f"{name}_loc")
                for j in range(NCH):
                    nc.scalar.copy(loc[:, j * CH:(j + 1) * CH], s1_ps[j][:])
                    nc.scalar.copy(loc[:, T + j * CH:T + (j + 1) * CH],
                                   s2_ps[j][:])
                cin = dpool.tile([1, 2 * T], fp32, name=f"{name}_cin")
                cout = dpool.tile([1, 2 * T], fp32, name=f"{name}_cout")
                nc.sync.dma_start(cin[:], loc[:])
                nc.gpsimd.collective_compute(
                    "AllReduce", mybir.AluOpType.add,
                    replica_groups=[list(range(NCORES))],
                    ins=[cin.opt()], outs=[cout.opt()],
                )
                glob = pool.tile([1, 2 * T], fp32, name=f"{name}_glob")
                nc.sync.dma_start(glob[:], cout[:])
                return glob[:, 0:T], glob[:, T:2 * T]

            def affine_chunked(dst_ap, src_ap, sc_bc, bi_bc):
                """dst = src * sc_bc + bi_bc, chunked (fp32 temps)."""
                for j in range(NCH):
                    sl = slice(j * CH, (j + 1) * CH)
                    tmp = spool.tile([P, CH], fp32, tag="ntmp", bufs=3,
                                     name="ntmp")
                    nc.vector.tensor_mul(tmp[:], src_ap[:, sl], sc_bc[:, sl])
                    nc.vector.tensor_add(dst_ap[:, sl], tmp[:], bi_bc[:, sl])

            u1 = []     # created at phase 4 (first use) to free SBUF earlier
            o_nrm = []  # created at phase 3

            with tc.tile_pool(name="onrm", bufs=1) as opool:
                with tc.tile_pool(name="hT", bufs=1) as hpool:
                    hT = [hpool.tile([P, T], bf16, tag=f"h{k}", name=f"hT_{k}")
                          for k in range(CT)]
                    # ================= Phase 1: LN1 =================
                    with tc.tile_pool(name="p1", bufs=1) as p1:
                        s1_ps = [pps.tile([1, CH], fp32, tag="st", bufs=4,
                                          name=f"ln1s1_{j}") for j in range(NCH)]
                        s2_ps = [pps.tile([1, CH], fp32, tag="st", bufs=4,
                                          name=f"ln1s2_{j}") for j in range(NCH)]
                        x_sb = []
                        for k in range(CT):
                            xk = p1.tile([P, T], fp32, tag=f"x{k}", name=f"x_{k}")
                            nc.sync.dma_start(xk[:], xT_d[ts(k, P), :])
                            x_sb.append(xk)
                            stats_accum(xk, s1_ps, s2_ps, first=(k == 0),
                                        last=(k == CT - 1))
                        s1r = rows_from_ps(p1, s1_ps, "ln1_s1")
                        s2r = rows_from_ps(p1, s2_ps, "ln1_s2")
                        a_row, b_row = norm_rows(p1, s1r[:], s2r[:], C, "ln1")
                        a_bc = broadcast_row(p1, a_row, "ln1_abc")
                        b_bc = broadcast_row(p1, b_row, "ln1_bbc")
                        for k in range(CT):
                            affine_chunked(hT[k], x_sb[k], a_bc, b_bc)

                    # ================= Phase 2: V for all heads ============
                    with tc.tile_pool(name="vall", bufs=1) as vpool:
                        Vall = [vpool.tile([P, C], bf16, tag=f"v{s}",
                                           name=f"V_{s}") for s in range(ST)]
                        with tc.tile_pool(name="wv", bufs=1) as wvpool:
                            wv_sb = []
                            for k in range(CT):
                                wvk = wvpool.tile([P, C], bf16, tag=f"wv{k}",
                                                  name=f"wv_{k}")
                                nc.sync.dma_start(wvk[:], wv_d[ts(k, P), :])
                                wv_sb.append(wvk)
                            for s in range(ST):
                                for n in range(C // CH):
                                    vps = ppw.tile([P, CH], fp32, tag="w",
                                                   name=f"v_ps_{s}_{n}")
                                    for k in range(CT):
                                        nc.tensor.matmul(
                                            vps[:], hT[k][:, ts(s, P)],
                                            wv_sb[k][:, ts(n, CH)],
                                            start=(k == 0), stop=(k == CT - 1))
                                    nc.scalar.copy(Vall[s][:, ts(n, CH)], vps[:])

                        # ============ Phase 3: per-head attention ==========
                        with tc.tile_pool(name="p3", bufs=1) as p3:
                            for h in range(H):
                                o_nrm.append(opool.tile(
                                    [P, T], bf16, tag=f"o{h}", name=f"on_{h}"))
                                wqh = p3.tile([P, CT, P], bf16, tag="wqh",
                                              bufs=2, name=f"wqh_{h}")
                                nc.sync.dma_start(wqh[:], wq_r[:, :, ts(h, P)])
                                wkh = p3.tile([P, CT, P], bf16, tag="wkh",
                                              bufs=2, name=f"wkh_{h}")
                                nc.sync.dma_start(wkh[:], wk_r[:, :, ts(h, P)])
                                qT = p3.tile([P, T], bf16, tag="qT", bufs=2,
                                             name=f"qT_{h}")
                                kT = p3.tile([P, T], bf16, tag="kT", bufs=2,
                                             name=f"kT_{h}")
                                for j in range(NCH):
                                    sl = slice(j * CH, (j + 1) * CH)
                                    qps = ppw.tile([P, CH], fp32, tag="w",
                                                   name=f"q_ps_{h}_{j}")
                                    for k in range(CT):
                                        nc.tensor.matmul(qps[:], wqh[:, k, :],
                                                         hT[k][:, sl],
                                                         start=(k == 0),
                                                         stop=(k == CT - 1))
                                    nc.scalar.activation(qT[:, sl], qps[:],
                                                         AF.Identity,
                                                         bias=bq_sb[:, h:h + 1],
                                                         scale=1.0)
                                    kps = ppw.tile([P, CH], fp32, tag="w",
                                                   name=f"k_ps_{h}_{j}")
                                    for k in range(CT):
                                        nc.tensor.matmul(kps[:], wkh[:, k, :],
                                                         hT[k][:, sl],
                                                         start=(k == 0),
                                                         stop=(k == CT - 1))
                                    nc.scalar.copy(kT[:, sl], kps[:])
                                # scores + exp (causal: s-tile covers t >= s*P)
                                aT = []
                                for s in range(ST):
                                    at = p3.tile([P, T], bf16, tag=f"a{s}",
                                                 bufs=1, name=f"aT_{h}_{s}")
                                    aT.append(at)
                                    for j in range(NCH):
                                        lo = max(j * CH, s * P)
                                        hi = (j + 1) * CH
                                        if lo >= hi:
                                            continue
                                        sps = ppw.tile([P, CH], fp32, tag="w",
                                                       name=f"s_ps_{h}_{s}_{j}")
                                        nc.tensor.matmul(sps[:, :hi - lo],
                                                         kT[:, ts(s, P)],
                                                         qT[:, lo:hi],
                                                         start=True, stop=True)
                                        nc.scalar.activation(at[:, lo:hi],
                                                             sps[:, :hi - lo],
                                                             AF.Exp)
                                    nc.vector.tensor_mul(at[:, ts(s, P)],
                                                         at[:, ts(s, P)],
                                                         trimask[:])
                                # denominators (ones-matmul over s tiles)
                                den_ps = [pps.tile([1, CH], fp32, tag="st",
                                                   bufs=4, name=f"dn_{h}_{j}")
                                          for j in range(NCH)]
                                for j in range(NCH):
                                    smax = min(ST, 4 * (j + 1))
                                    for s in range(smax):
                                        lo = max(0, s * P - j * CH)
                                        nc.tensor.matmul(
                                            den_ps[j][:, lo:CH], ones_bf[:],
                                            aT[s][:, j * CH + lo:(j + 1) * CH],
                                            start=(s == 0), stop=(s == smax - 1))
                                den = p3.tile([1, T], fp32, tag="den", bufs=2,
                                              name=f"den_{h}")
                                for j in range(NCH):
                                    nc.scalar.copy(den[:, ts(j, CH)],
                                                   den_ps[j][:])
                                rrow = p3.tile([1, T], fp32, tag="rrow", bufs=2,
                                               name=f"rrow_{h}")
                                nc.vector.reciprocal(rrow[:], den[:])
                                r_bc = broadcast_row(p3, rrow, f"rbc_{h}",
                                                     tag="rbc", bufs=2)
                                # attention @ V, then normalize
                                for j in range(NCH):
                                    smax = min(ST, 4 * (j + 1))
                                    ops_ = ppw.tile([P, CH], fp32, tag="w",
                                                    name=f"o_ps_{h}_{j}")
                                    for s in range(smax):
                                        lo = max(0, s * P - j * CH)
                                        nc.tensor.matmul(
                                            ops_[:, lo:CH],
                                            Vall[s][:, ts(h, P)],
                                            aT[s][:, j * CH + lo:(j + 1) * CH],
                                            start=(s == 0), stop=(s == smax - 1))
                                    sl = slice(j * CH, (j + 1) * CH)
                                    nc.vector.tensor_mul(o_nrm[h][:, sl],
                                                         ops_[:], r_bc[:, sl])

                # hT pool closed; Phase 4: out-proj + residual + BN1 stats
                t1_ps = [pps.tile([1, CH], fp32, tag="st", bufs=4,
                                  name=f"bn1s1_{j}") for j in range(NCH)]
                t2_ps = [pps.tile([1, CH], fp32, tag="st", bufs=4,
                                  name=f"bn1s2_{j}") for j in range(NCH)]
                with tc.tile_pool(name="p4", bufs=1) as p4:
                    for k in range(CT):
                        u1.append(u1pool.tile([P, T], fp32, tag=f"u{k}",
                                              name=f"u1_{k}"))
                        wok = p4.tile([P, H, P], bf16, tag="wok", bufs=2,
                                      name=f"wok_{k}")
                        nc.sync.dma_start(wok[:], wo_r[:, :, ts(k, P)])
                        x2k = p4.tile([P, T], fp32, tag="x2", bufs=3,
                                      name=f"x2_{k}")
                        nc.sync.dma_start(x2k[:], xT_d[ts(k, P), :])
                        for j in range(NCH):
                            sl = slice(j * CH, (j + 1) * CH)
                            saps = ppw.tile([P, CH], fp32, tag="w",
                                            name=f"sa_ps_{k}_{j}")
                            for hh in range(H):
                                nc.tensor.matmul(saps[:], wok[:, hh, :],
                                                 o_nrm[hh][:, sl],
                                                 start=(hh == 0),
                                                 stop=(hh == H - 1))
                            nc.vector.scalar_tensor_tensor(
                                out=u1[k][:, sl], in0=saps[:],
                                scalar=bo_sb[:, k:k + 1], in1=x2k[:, sl],
                                op0=OP.add, op1=OP.add)
                        stats_accum(u1[k], t1_ps, t2_ps,
                                    first=(k == 0), last=(k == CT - 1))

            # ================= Phase 5: BN1 + LN2 =================
            with tc.tile_pool(name="h2T", bufs=1) as h2pool:
                h2T = [h2pool.tile([P, T], bf16, tag=f"h2{k}", name=f"h2_{k}")
                       for k in range(CT)]
                b1s_ps = [pps.tile([1, CH], fp32, tag="st", bufs=4,
                                   name=f"bn2s1_{j}") for j in range(NCH)]
                b2s_ps = [pps.tile([1, CH], fp32, tag="st", bufs=4,
                                   name=f"bn2s2_{j}") for j in range(NCH)]
                with tc.tile_pool(name="p5", bufs=1) as p5:
                    bn1g_sb = p5.tile([1, T], fp32, name="bn1g_sb")
                    nc.sync.dma_start(bn1g_sb[:], bn1g_d[:])
                    bn1b_sb = p5.tile([1, T], fp32, name="bn1b_sb")
                    nc.sync.dma_start(bn1b_sb[:], bn1b_d[:])
                    g1, g2 = allreduce_rows(p5, t1_ps, t2_ps, "bn1")
                    sc_row, bi_row = norm_rows(p5, g1, g2, NBC, "bn1",
                                               g_row=bn1g_sb, b_row=bn1b_sb)
                    sc_bc = broadcast_row(p5, sc_row, "bn1_scbc")
                    bi_bc = broadcast_row(p5, bi_row, "bn1_bibc")
                    l1_ps = [pps.tile([1, CH], fp32, tag="st", bufs=4,
                                      name=f"ln2s1_{j}") for j in range(NCH)]
                    l2_ps = [pps.tile([1, CH], fp32, tag="st", bufs=4,
                                      name=f"ln2s2_{j}") for j in range(NCH)]
                    for k in range(CT):
                        affine_chunked(u1[k], u1[k], sc_bc, bi_bc)
                        stats_accum(u1[k], l1_ps, l2_ps, first=(k == 0),
                                    last=(k == CT - 1))
                    ls1 = rows_from_ps(p5, l1_ps, "ln2_s1")
                    ls2 = rows_from_ps(p5, l2_ps, "ln2_s2")
                    a2_row, b2_row = norm_rows(p5, ls1[:], ls2[:], C, "ln2")
                    a2_bc = broadcast_row(p5, a2_row, "ln2_abc")
                    b2_bc = broadcast_row(p5, b2_row, "ln2_bbc")
                    for k in range(CT):
                        affine_chunked(h2T[k], u1[k], a2_bc, b2_bc)

                # ================= Phase 6: FFN =================
                with tc.tile_pool(name="p6", bufs=1) as p6:
                    for j in range(NCH):
                        sl = slice(j * CH, (j + 1) * CH)
                        z = []
                        for f in range(FT):
                            w1f = p6.tile([P, CT, P], bf16, tag="w1f", bufs=2,
                                          name=f"w1f_{j}_{f}")
                            nc.sync.dma_start(w1f[:], w1_r[:, :, ts(f, P)])
                            zps = ppw.tile([P, CH], fp32, tag="w",
                                           name=f"z_ps_{j}_{f}")
                            for k in range(CT):
                                nc.tensor.matmul(zps[:], w1f[:, k, :],
                                                 h2T[k][:, sl],
                                                 start=(k == 0),
                                                 stop=(k == CT - 1))
                            zf = p6.tile([P, CH], bf16, tag=f"z{f}",
                                         name=f"z_{j}_{f}")
                            nc.scalar.activation(zf[:], zps[:], AF.Relu,
                                                 bias=b1_sb[:, f:f + 1],
                                                 scale=1.0)
                            z.append(zf)
                        for k in range(CT):
                            w2k = p6.tile([P, FT, P], bf16, tag="w2k", bufs=2,
                                          name=f"w2k_{j}_{k}")
                            nc.sync.dma_start(w2k[:], w2_r[:, :, ts(k, P)])
                            yps = ppw.tile([P, CH], fp32, tag="w",
                                           name=f"y_ps_{j}_{k}")
                            for f in range(FT):
                                nc.tensor.matmul(yps[:], w2k[:, f, :], z[f][:],
                                                 start=(f == 0),
                                                 stop=(f == FT - 1))
                            nc.vector.scalar_tensor_tensor(
                                out=u1[k][:, sl], in0=yps[:],
                                scalar=b2_sb[:, k:k + 1], in1=u1[k][:, sl],
                                op0=OP.add, op1=OP.add)
                            cbf = spool.tile([P, CH], bf16, tag="cast_bf",
                                             bufs=3, name="cbf2")
                            nc.vector.tensor_copy(cbf[:], u1[k][:, sl])
                            csq = spool.tile([P, CH], bf16, tag="cast_sq",
                                             bufs=3, name="csq2")
                            nc.scalar.square(csq[:], u1[k][:, sl])
                            nc.tensor.matmul(b1s_ps[j][:], ones_bf[:], cbf[:],
                                             start=(k == 0), stop=(k == CT - 1))
                            nc.tensor.matmul(b2s_ps[j][:], ones_bf[:], csq[:],
                                             start=(k == 0), stop=(k == CT - 1))

            # ================= Phase 7: BN2 + output =================
            with tc.tile_pool(name="p7", bufs=1) as p7:
                bn2g_sb = p7.tile([1, T], fp32, name="bn2g_sb")
                nc.sync.dma_start(bn2g_sb[:], bn2g_d[:])
                bn2b_sb = p7.tile([1, T], fp32, name="bn2b_sb")
                nc.sync.dma_start(bn2b_sb[:], bn2b_d[:])
                h1, h2 = allreduce_rows(p7, b1s_ps, b2s_ps, "bn2")
                sc2_row, bi2_row = norm_rows(p7, h1, h2, NBC, "bn2",
                                             g_row=bn2g_sb, b_row=bn2b_sb)
                sc2_bc = broadcast_row(p7, sc2_row, "bn2_scbc")
                bi2_bc = broadcast_row(p7, bi2_row, "bn2_bibc")
                for k in range(CT):
                    for j in range(NCH):
                        sl = slice(j * CH, (j + 1) * CH)
                        tmp = spool.tile([P, CH], fp32, tag="ntmp", bufs=3,
                                         name="ytmp")
                        nc.vector.tensor_mul(tmp[:], u1[k][:, sl],
                                             sc2_bc[:, sl])
                        yk = spool.tile([P, CH], fp32, tag="yout", bufs=3,
                                        name=f"y_{k}_{j}")
                        nc.vector.tensor_add(yk[:], tmp[:], bi2_bc[:, sl])
                        nc.sync.dma_start(yT_d[ts(k, P), sl], yk[:])

    nc.compile()
    return nc


def _get_program():
    global _PROG
    if _PROG is None:
        _PROG = _build()
    return _PROG


def _prep_shared(inputs):
    """Host-side weight folding; identical for every core."""
    f32 = np.float32
    bf16 = ml_dtypes.bfloat16
    wq = np.asarray(inputs["wq"], f32)      # (H, C, D)
    wk = np.asarray(inputs["wk"], f32)
    wv = np.asarray(inputs["wv"], f32)
    wo = np.asarray(inputs["wo"], f32)      # (C, C)
    bo = np.asarray(inputs["bo"], f32)      # (C,)
    g1 = np.asarray(inputs["ln1_g"], f32)
    b1n = np.asarray(inputs["ln1_b"], f32)
    g2 = np.asarray(inputs["ln2_g"], f32)
    b2n = np.asarray(inputs["ln2_b"], f32)
    w1 = np.asarray(inputs["w1"], f32)      # (C, F)
    b1 = np.asarray(inputs["b1"], f32)      # (F,)
    w2 = np.asarray(inputs["w2"], f32)      # (F, C)
    b2 = np.asarray(inputs["b2"], f32)      # (C,)

    dscale = f32(D) ** f32(-0.5)
    # fold ln1 affine into qkv projections; q also takes 1/sqrt(D)
    wq2 = (wq * g1[None, :, None] * dscale).transpose(1, 0, 2).reshape(C, C)
    wk2 = (wk * g1[None, :, None]).transpose(1, 0, 2).reshape(C, C)
    wv2 = (wv * g1[None, :, None]).transpose(1, 0, 2).reshape(C, C)
    bq = (np.einsum("c,hcd->hd", b1n, wq) * dscale).reshape(C)
    bv = np.einsum("c,hcd->hd", b1n, wv).reshape(C)
    # k-side bias cancels in softmax (constant per row); v bias folds into bo
    bo2 = bo + bv @ wo
    w1f = g2[:, None] * w1
    b1f = b1 + b2n @ w1

    def cols(v, n):  # (n*P,) -> (P, n) with [p, i] = v[i*P + p]
        return np.ascontiguousarray(v.reshape(n, P).T, dtype=f32)

    def row(v):
        return np.ascontiguousarray(v.reshape(1, T), dtype=f32)

    return dict(
        wq=wq2.astype(bf16), wk=wk2.astype(bf16), wv=wv2.astype(bf16),
        bq=cols(bq, H), wo=wo.astype(bf16), bo=cols(bo2, CT),
        w1=w1f.astype(bf16), b1=cols(b1f, FT),
        w2=w2.astype(bf16), b2=cols(b2, CT),
        bn1g=row(np.asarray(inputs["bn1_g"], f32)),
        bn1b=row(np.asarray(inputs["bn1_b"], f32)),
        bn2g=row(np.asarray(inputs["bn2_g"], f32)),
        bn2b=row(np.asarray(inputs["bn2_b"], f32)),
    )


def _run(inputs, trace=False):
    from concourse import bass_utils
    nc = _get_program()
    x = np.asarray(inputs["x"], np.float32)
    shared = _prep_shared(inputs)
    in_maps = []
    for b in range(B):
        m = dict(shared)
        m["xT"] = np.ascontiguousarray(x[b].T)
        in_maps.append(m)
    res = bass_utils.run_bass_kernel_spmd(
        nc, in_maps, core_ids=list(range(NCORES)), trace=trace)
    out = np.stack([res.results[b]["yT"].T for b in range(B)]).astype(np.float32)
    return out, res


def kernel(**inputs):
    out, _ = _run(inputs, trace=False)
    return out


# revision 7
# speedup vs baseline: 1.0387x; 1.0387x over previous
"""Trainium2 Bass kernel for nn_Block_50113678410401 (dense transformer block).

Strategy: data-parallel over the batch axis (B=8 -> 8 NeuronCores, one batch
element per core). All on-chip activations live in "layout A": feature axis on
SBUF partitions, token axis (T) on the free dimension, so no on-chip
transposes are needed (host pre-transposes x and post-transposes the output).

Per core:
  LN1 (stats via ones-matmul over partitions), per-head causal attention
  (no-max-sub exp softmax, denominator via ones-matmul, normalization via
  K=1 broadcast matmul), output projection + residual, BatchNorm over (B,C)
  with a cross-core AllReduce of (sum, sumsq) per T channel, LN2, FFN
  (C -> 4C -> relu -> C), residual, second BatchNorm (second AllReduce).

All big matmuls run in bf16 with fp32 PSUM accumulation; statistics,
softmax, residuals and normalizations are fp32. Weights arrive host-pretiled
so every weight DMA is contiguous per partition.

LayerNorm/projection affine parameters are folded into the weights on the
host: wq' = diag(ln1_g) wq / sqrt(D) (q also carries 1/sqrt(D)), k-side bias
drops out of softmax by shift invariance, v-side bias is folded into the
output-projection bias, ln2 affine is folded into w1/b1.
"""

import numpy as np
import ml_dtypes

B, T, C, H, D = 8, 1024, 1536, 12, 128
F = 4 * C            # 6144
P = 128
CT = C // P          # 12 c-tiles
FT = F // P          # 48 f-tiles
ST = T // P          # 8 s-tiles
CH = 512             # matmul free-dim chunk
NCH = T // CH        # 2 chunks
EPS = 1e-5
NCORES = 8
NBC = B * C          # BatchNorm count over (B, C)

_PROG = None


def _build():
    import concourse.bass as bass
    import concourse.mybir as mybir
    import concourse.tile as tile
    from concourse import bacc
    from concourse.masks import make_upper_triangular

    fp32 = mybir.dt.float32
    bf16 = mybir.dt.bfloat16
    AF = mybir.ActivationFunctionType
    OP = mybir.AluOpType
    ts = bass.ts

    nc = bacc.Bacc("TRN2", target_bir_lowering=False, debug=False,
                   enable_asserts=True, num_devices=NCORES)

    # ---- DRAM I/O (weights host-pretiled for contiguous DMA) ----
    xT_d = nc.dram_tensor("xT", (C, T), fp32, kind="ExternalInput").ap()
    wq_d = nc.dram_tensor("wq", (H, P, CT, P), bf16, kind="ExternalInput").ap()
    wk_d = nc.dram_tensor("wk", (H, P, CT, P), bf16, kind="ExternalInput").ap()
    wv_d = nc.dram_tensor("wv", (C, C), bf16, kind="ExternalInput").ap()
    bq_d = nc.dram_tensor("bq", (P, H), fp32, kind="ExternalInput").ap()
    wo_d = nc.dram_tensor("wo", (CT, P, H, P), bf16, kind="ExternalInput").ap()
    bo_d = nc.dram_tensor("bo", (P, CT), fp32, kind="ExternalInput").ap()
    w1_d = nc.dram_tensor("w1", (FT, P, CT, P), bf16, kind="ExternalInput").ap()
    b1_d = nc.dram_tensor("b1", (P, FT), fp32, kind="ExternalInput").ap()
    w2_d = nc.dram_tensor("w2", (CT, P, FT, P), bf16, kind="ExternalInput").ap()
    b2_d = nc.dram_tensor("b2", (P, CT), fp32, kind="ExternalInput").ap()
    bn1g_d = nc.dram_tensor("bn1g", (1, T), fp32, kind="ExternalInput").ap()
    bn1b_d = nc.dram_tensor("bn1b", (1, T), fp32, kind="ExternalInput").ap()
    bn2g_d = nc.dram_tensor("bn2g", (1, T), fp32, kind="ExternalInput").ap()
    bn2b_d = nc.dram_tensor("bn2b", (1, T), fp32, kind="ExternalInput").ap()
    yT_d = nc.dram_tensor("yT", (C, T), fp32, kind="ExternalOutput").ap()

    with tile.TileContext(nc) as tc:
        with tc.tile_pool(name="const", bufs=1) as cpool, \
             tc.tile_pool(name="scratch", bufs=1) as spool, \
             tc.tile_pool(name="u1p", bufs=1) as u1pool, \
             tc.tile_pool(name="ppw", bufs=6, space="PSUM") as ppw, \
             tc.tile_pool(name="pps", bufs=2, space="PSUM") as pps, \
             tc.tile_pool(name="dram", bufs=1, space="DRAM") as dpool:

            # ---- constants ----
            ones_bf = cpool.tile([P, 1], bf16, name="ones_bf")
            nc.vector.memset(ones_bf[:], 1.0)
            ones1f = cpool.tile([1, P], fp32, name="ones1f")
            nc.vector.memset(ones1f[:], 1.0)
            trimask = cpool.tile([P, P], bf16, name="trimask")
            make_upper_triangular(nc, trimask[:], val=1.0, diag=True)
            bq_sb = cpool.tile([P, H], fp32, name="bq_sb")
            nc.sync.dma_start(bq_sb[:], bq_d[:])
            bo_sb = cpool.tile([P, CT], fp32, name="bo_sb")
            nc.sync.dma_start(bo_sb[:], bo_d[:])
            b1_sb = cpool.tile([P, FT], fp32, name="b1_sb")
            nc.sync.dma_start(b1_sb[:], b1_d[:])
            b2_sb = cpool.tile([P, CT], fp32, name="b2_sb")
            nc.sync.dma_start(b2_sb[:], b2_d[:])

            # ---- helpers ----
            # Packed stat psum tile: row 0 accumulates sum, row 32 sumsq.
            def stat_tiles(name):
                return [pps.tile([P, CH], fp32, tag="st", bufs=2,
                                 name=f"{name}_{j}") for j in range(NCH)]

            def stats_chunk(src_ap, stp_j, first, last):
                """Ones-matmul partial sums of src chunk (fp32 (P,CH)) and its
                square into packed stat rows."""
                cbf = spool.tile([P, CH], bf16, tag="cast_bf", bufs=3,
                                 name="cbf")
                nc.vector.tensor_copy(cbf[:], src_ap)
                csq = spool.tile([P, CH], bf16, tag="cast_sq", bufs=3,
                                 name="csq")
                nc.scalar.square(csq[:], src_ap)
                nc.tensor.matmul(stp_j[0:1, :], ones_bf[:], cbf[:],
                                 start=first, stop=last)
                nc.tensor.matmul(stp_j[32:33, :], ones_bf[:], csq[:],
                                 start=first, stop=last)

            def stat_rows(pool, stp, name):
                """Copy packed stat psums into (sum, sumsq) (1,T) rows."""
                s1 = pool.tile([1, T], fp32, name=f"{name}_s1")
                s2 = pool.tile([1, T], fp32, name=f"{name}_s2")
                for j in range(NCH):
                    nc.scalar.copy(s1[:, j * CH:(j + 1) * CH], stp[j][0:1, :])
                    nc.scalar.copy(s2[:, j * CH:(j + 1) * CH], stp[j][32:33, :])
                return s1, s2

            def norm_rows(pool, s1row, s2row, count, name,
                          g_row=None, b_row=None):
                """scale = (1/sqrt(var+eps)) * g ; bias = -mean*scale + b."""
                m = pool.tile([1, T], fp32, name=f"{name}_m")
                nc.vector.tensor_scalar_mul(m[:], s1row, 1.0 / count)
                v = pool.tile([1, T], fp32, name=f"{name}_v")
                nc.vector.tensor_scalar_mul(v[:], s2row, 1.0 / count)
                msq = pool.tile([1, T], fp32, name=f"{name}_msq")
                nc.vector.tensor_mul(msq[:], m[:], m[:])
                nc.vector.tensor_sub(v[:], v[:], msq[:])
                nc.vector.tensor_scalar_add(v[:], v[:], EPS)
                nc.scalar.sqrt(v[:], v[:])
                scale = pool.tile([1, T], fp32, name=f"{name}_scale")
                nc.vector.reciprocal(scale[:], v[:])
                if g_row is not None:
                    nc.vector.tensor_mul(scale[:], scale[:], g_row[:])
                bias = pool.tile([1, T], fp32, name=f"{name}_bias")
                nc.vector.tensor_mul(bias[:], m[:], scale[:])
                nc.vector.tensor_scalar_mul(bias[:], bias[:], -1.0)
                if b_row is not None:
                    nc.vector.tensor_add(bias[:], bias[:], b_row[:])
                return scale, bias

            def broadcast_row(pool, row, name, tag="bc", bufs=2):
                """(1, T) fp32 -> (P, T) fp32 via K=1 matmul."""
                bc = pool.tile([P, T], fp32, tag=tag, bufs=bufs, name=name)
                for j in range(NCH):
                    sl = slice(j * CH, (j + 1) * CH)
                    ps = ppw.tile([P, CH], fp32, tag="w", name=f"{name}_ps")
                    nc.tensor.matmul(ps[:], ones1f[:], row[:, sl],
                                     start=True, stop=True)
                    nc.scalar.copy(bc[:, sl], ps[:])
                return bc

            def allreduce_rows(pool, stp, name):
                """Assemble (sum, sumsq) rows, AllReduce-add across cores."""
                loc = pool.tile([1, 2 * T], fp32, name=f"{name}_loc")
                for j in range(NCH):
                    nc.scalar.copy(loc[:, j * CH:(j + 1) * CH], stp[j][0:1, :])
                    nc.scalar.copy(loc[:, T + j * CH:T + (j + 1) * CH],
                                   stp[j][32:33, :])
                cin = dpool.tile([1, 2 * T], fp32, name=f"{name}_cin")
                cout = dpool.tile([1, 2 * T], fp32, name=f"{name}_cout")
                nc.sync.dma_start(cin[:], loc[:])
                nc.gpsimd.collective_compute(
                    "AllReduce", mybir.AluOpType.add,
                    replica_groups=[list(range(NCORES))],
                    ins=[cin.opt()], outs=[cout.opt()],
                )
                glob = pool.tile([1, 2 * T], fp32, name=f"{name}_glob")
                nc.sync.dma_start(glob[:], cout[:])
                return glob[:, 0:T], glob[:, T:2 * T]

            def affine_chunk(dst_ap, src_ap, sc_ap, bi_ap):
                """dst = src * sc + bi on one (P, CH) chunk."""
                tmp = spool.tile([P, CH], fp32, tag="ntmp", bufs=3,
                                 name="ntmp")
                nc.vector.tensor_mul(tmp[:], src_ap, sc_ap)
                nc.vector.tensor_add(dst_ap, tmp[:], bi_ap)

            u1 = []     # created at phase 4 (first use)
            o_nrm = []  # created at phase 3

            with tc.tile_pool(name="onrm", bufs=1) as opool:
                with tc.tile_pool(name="hT", bufs=1) as hpool:
                    hT = [hpool.tile([P, T], bf16, tag=f"h{k}", name=f"hT_{k}")
                          for k in range(CT)]
                    # ================= Phase 1: LN1 =================
                    with tc.tile_pool(name="p1", bufs=1) as p1:
                        stp = stat_tiles("ln1")
                        x_sb = []
                        for k in range(CT):
                            xk = p1.tile([P, T], fp32, tag=f"x{k}", name=f"x_{k}")
                            nc.sync.dma_start(xk[:], xT_d[ts(k, P), :])
                            x_sb.append(xk)
                            for j in range(NCH):
                                sl = slice(j * CH, (j + 1) * CH)
                                stats_chunk(xk[:, sl], stp[j], k == 0,
                                            k == CT - 1)
                        s1r, s2r = stat_rows(p1, stp, "ln1")
                        a_row, b_row = norm_rows(p1, s1r[:], s2r[:], C, "ln1")
                        a_bc = broadcast_row(p1, a_row, "ln1_abc")
                        b_bc = broadcast_row(p1, b_row, "ln1_bbc")
                        for j in range(NCH):
                            sl = slice(j * CH, (j + 1) * CH)
                            for k in range(CT):
                                affine_chunk(hT[k][:, sl], x_sb[k][:, sl],
                                             a_bc[:, sl], b_bc[:, sl])

                    # ================= Phase 2: V for all heads ============
                    with tc.tile_pool(name="vall", bufs=1) as vpool:
                        Vall = [vpool.tile([P, C], bf16, tag=f"v{s}",
                                           name=f"V_{s}") for s in range(ST)]
                        with tc.tile_pool(name="wv", bufs=1) as wvpool:
                            wv_sb = []
                            for k in range(CT):
                                wvk = wvpool.tile([P, C], bf16, tag=f"wv{k}",
                                                  name=f"wv_{k}")
                                nc.sync.dma_start(wvk[:], wv_d[ts(k, P), :])
                                wv_sb.append(wvk)
                            for s in range(ST):
                                for n in range(C // CH):
                                    vps = ppw.tile([P, CH], fp32, tag="w",
                                                   name=f"v_ps_{s}_{n}")
                                    for k in range(CT):
                                        nc.tensor.matmul(
                                            vps[:], hT[k][:, ts(s, P)],
                                            wv_sb[k][:, ts(n, CH)],
                                            start=(k == 0), stop=(k == CT - 1))
                                    nc.scalar.copy(Vall[s][:, ts(n, CH)], vps[:])

                        # ============ Phase 3: per-head attention ==========
                        with tc.tile_pool(name="p3", bufs=1) as p3:
                            for h in range(H):
                                o_nrm.append(opool.tile(
                                    [P, T], bf16, tag=f"o{h}", name=f"on_{h}"))
                                wqh = p3.tile([P, CT, P], bf16, tag="wqh",
                                              bufs=2, name=f"wqh_{h}")
                                nc.sync.dma_start(wqh[:], wq_d[h])
                                wkh = p3.tile([P, CT, P], bf16, tag="wkh",
                                              bufs=2, name=f"wkh_{h}")
                                nc.sync.dma_start(wkh[:], wk_d[h])
                                qT = p3.tile([P, T], bf16, tag="qT", bufs=2,
                                             name=f"qT_{h}")
                                kT = p3.tile([P, T], bf16, tag="kT", bufs=2,
                                             name=f"kT_{h}")
                                for j in range(NCH):
                                    sl = slice(j * CH, (j + 1) * CH)
                                    qps = ppw.tile([P, CH], fp32, tag="w",
                                                   name=f"q_ps_{h}_{j}")
                                    for k in range(CT):
                                        nc.tensor.matmul(qps[:], wqh[:, k, :],
                                                         hT[k][:, sl],
                                                         start=(k == 0),
                                                         stop=(k == CT - 1))
                                    nc.scalar.activation(qT[:, sl], qps[:],
                                                         AF.Identity,
                                                         bias=bq_sb[:, h:h + 1],
                                                         scale=1.0)
                                    kps = ppw.tile([P, CH], fp32, tag="w",
                                                   name=f"k_ps_{h}_{j}")
                                    for k in range(CT):
                                        nc.tensor.matmul(kps[:], wkh[:, k, :],
                                                         hT[k][:, sl],
                                                         start=(k == 0),
                                                         stop=(k == CT - 1))
                                    nc.scalar.copy(kT[:, sl], kps[:])
                                # scores + exp (causal: s-tile covers t >= s*P)
                                aT = []
                                for s in range(ST):
                                    at = p3.tile([P, T], bf16, tag=f"a{s}",
                                                 bufs=1, name=f"aT_{h}_{s}")
                                    aT.append(at)
                                    for j in range(NCH):
                                        lo = max(j * CH, s * P)
                                        hi = (j + 1) * CH
                                        if lo >= hi:
                                            continue
                                        sps = ppw.tile([P, CH], fp32, tag="w",
                                                       name=f"s_ps_{h}_{s}_{j}")
                                        nc.tensor.matmul(sps[:, :hi - lo],
                                                         kT[:, ts(s, P)],
                                                         qT[:, lo:hi],
                                                         start=True, stop=True)
                                        nc.scalar.activation(at[:, lo:hi],
                                                             sps[:, :hi - lo],
                                                             AF.Exp)
                                    nc.vector.tensor_mul(at[:, ts(s, P)],
                                                         at[:, ts(s, P)],
                                                         trimask[:])
                                # denominators: packed psum, row 0 (j=0)
                                # and row 32 (j=1)
                                den_ps = pps.tile([P, CH], fp32, tag="st",
                                                  bufs=2, name=f"dn_{h}")
                                for j in range(NCH):
                                    r0 = 32 * j
                                    smax = min(ST, 4 * (j + 1))
                                    for s in range(smax):
                                        lo = max(0, s * P - j * CH)
                                        nc.tensor.matmul(
                                            den_ps[r0:r0 + 1, lo:CH],
                                            ones_bf[:],
                                            aT[s][:, j * CH + lo:(j + 1) * CH],
                                            start=(s == 0), stop=(s == smax - 1))
                                den = p3.tile([1, T], fp32, tag="den", bufs=2,
                                              name=f"den_{h}")
                                for j in range(NCH):
                                    nc.scalar.copy(den[:, ts(j, CH)],
                                                   den_ps[32 * j:32 * j + 1, :])
                                rrow = p3.tile([1, T], fp32, tag="rrow", bufs=2,
                                               name=f"rrow_{h}")
                                nc.vector.reciprocal(rrow[:], den[:])
                                r_bc = broadcast_row(p3, rrow, f"rbc_{h}",
                                                     tag="rbc", bufs=2)
                                # attention @ V, then normalize
                                for j in range(NCH):
                                    smax = min(ST, 4 * (j + 1))
                                    ops_ = ppw.tile([P, CH], fp32, tag="w",
                                                    name=f"o_ps_{h}_{j}")
                                    for s in range(smax):
                                        lo = max(0, s * P - j * CH)
                                        nc.tensor.matmul(
                                            ops_[:, lo:CH],
                                            Vall[s][:, ts(h, P)],
                                            aT[s][:, j * CH + lo:(j + 1) * CH],
                                            start=(s == 0), stop=(s == smax - 1))
                                    sl = slice(j * CH, (j + 1) * CH)
                                    nc.vector.tensor_mul(o_nrm[h][:, sl],
                                                         ops_[:], r_bc[:, sl])

                # hT closed; Phase 4: out-proj + residual + BN1 stats
                stp_bn1 = stat_tiles("bn1")
                with tc.tile_pool(name="p4", bufs=1) as p4:
                    for k in range(CT):
                        u1.append(u1pool.tile([P, T], fp32, tag=f"u{k}",
                                              name=f"u1_{k}"))
                        wok = p4.tile([P, H, P], bf16, tag="wok", bufs=2,
                                      name=f"wok_{k}")
                        nc.sync.dma_start(wok[:], wo_d[k])
                        x2k = p4.tile([P, T], fp32, tag="x2", bufs=3,
                                      name=f"x2_{k}")
                        nc.sync.dma_start(x2k[:], xT_d[ts(k, P), :])
                        for j in range(NCH):
                            sl = slice(j * CH, (j + 1) * CH)
                            saps = ppw.tile([P, CH], fp32, tag="w",
                                            name=f"sa_ps_{k}_{j}")
                            for hh in range(H):
                                nc.tensor.matmul(saps[:], wok[:, hh, :],
                                                 o_nrm[hh][:, sl],
                                                 start=(hh == 0),
                                                 stop=(hh == H - 1))
                            nc.vector.scalar_tensor_tensor(
                                out=u1[k][:, sl], in0=saps[:],
                                scalar=bo_sb[:, k:k + 1], in1=x2k[:, sl],
                                op0=OP.add, op1=OP.add)
                            stats_chunk(u1[k][:, sl], stp_bn1[j],
                                        k == 0, k == CT - 1)

            # ================= Phase 5: BN1 + LN2 =================
            with tc.tile_pool(name="h2T", bufs=1) as h2pool:
                h2T = [h2pool.tile([P, T], bf16, tag=f"h2{k}", name=f"h2_{k}")
                       for k in range(CT)]
                with tc.tile_pool(name="p5", bufs=1) as p5:
                    bn1g_sb = p5.tile([1, T], fp32, name="bn1g_sb")
                    nc.sync.dma_start(bn1g_sb[:], bn1g_d[:])
                    bn1b_sb = p5.tile([1, T], fp32, name="bn1b_sb")
                    nc.sync.dma_start(bn1b_sb[:], bn1b_d[:])
                    g1, g2 = allreduce_rows(p5, stp_bn1, "bn1")
                    sc_row, bi_row = norm_rows(p5, g1, g2, NBC, "bn1",
                                               g_row=bn1g_sb, b_row=bn1b_sb)
                    sc_bc = broadcast_row(p5, sc_row, "bn1_scbc")
                    bi_bc = broadcast_row(p5, bi_row, "bn1_bibc")
                    stp_ln2 = stat_tiles("ln2")
                    for j in range(NCH):
                        sl = slice(j * CH, (j + 1) * CH)
                        for k in range(CT):
                            affine_chunk(u1[k][:, sl], u1[k][:, sl],
                                         sc_bc[:, sl], bi_bc[:, sl])
                            stats_chunk(u1[k][:, sl], stp_ln2[j],
                                        k == 0, k == CT - 1)
                    ls1, ls2 = stat_rows(p5, stp_ln2, "ln2")
                    a2_row, b2_row = norm_rows(p5, ls1[:], ls2[:], C, "ln2")
                    a2_bc = broadcast_row(p5, a2_row, "ln2_abc")
                    b2_bc = broadcast_row(p5, b2_row, "ln2_bbc")
                    for j in range(NCH):
                        sl = slice(j * CH, (j + 1) * CH)
                        for k in range(CT):
                            affine_chunk(h2T[k][:, sl], u1[k][:, sl],
                                         a2_bc[:, sl], b2_bc[:, sl])

                # ================= Phase 6: FFN =================
                stp_bn2 = stat_tiles("bn2")
                with tc.tile_pool(name="p6", bufs=1) as p6:
                    for j in range(NCH):
                        sl = slice(j * CH, (j + 1) * CH)
                        z = []
                        for f in range(FT):
                            w1f = p6.tile([P, CT, P], bf16, tag="w1f", bufs=2,
                                          name=f"w1f_{j}_{f}")
                            nc.sync.dma_start(w1f[:], w1_d[f])
                            zps = ppw.tile([P, CH], fp32, tag="w",
                                           name=f"z_ps_{j}_{f}")
                            for k in range(CT):
                                nc.tensor.matmul(zps[:], w1f[:, k, :],
                                                 h2T[k][:, sl],
                                                 start=(k == 0),
                                                 stop=(k == CT - 1))
                            zf = p6.tile([P, CH], bf16, tag=f"z{f}",
                                         name=f"z_{j}_{f}")
                            nc.scalar.activation(zf[:], zps[:], AF.Relu,
                                                 bias=b1_sb[:, f:f + 1],
                                                 scale=1.0)
                            z.append(zf)
                        for k in range(CT):
                            w2k = p6.tile([P, FT, P], bf16, tag="w2k", bufs=2,
                                          name=f"w2k_{j}_{k}")
                            nc.sync.dma_start(w2k[:], w2_d[k])
                            yps = ppw.tile([P, CH], fp32, tag="w",
                                           name=f"y_ps_{j}_{k}")
                            for f in range(FT):
                                nc.tensor.matmul(yps[:], w2k[:, f, :], z[f][:],
                                                 start=(f == 0),
                                                 stop=(f == FT - 1))
                            nc.vector.scalar_tensor_tensor(
                                out=u1[k][:, sl], in0=yps[:],
                                scalar=b2_sb[:, k:k + 1], in1=u1[k][:, sl],
                                op0=OP.add, op1=OP.add)
                            stats_chunk(u1[k][:, sl], stp_bn2[j],
                                        k == 0, k == CT - 1)

            # ================= Phase 7: BN2 + output =================
            with tc.tile_pool(name="p7", bufs=1) as p7:
                bn2g_sb = p7.tile([1, T], fp32, name="bn2g_sb")
                nc.sync.dma_start(bn2g_sb[:], bn2g_d[:])
                bn2b_sb = p7.tile([1, T], fp32, name="bn2b_sb")
                nc.sync.dma_start(bn2b_sb[:], bn2b_d[:])
                h1, h2 = allreduce_rows(p7, stp_bn2, "bn2")
                sc2_row, bi2_row = norm_rows(p7, h1, h2, NBC, "bn2",
                                             g_row=bn2g_sb, b_row=bn2b_sb)
                sc2_bc = broadcast_row(p7, sc2_row, "bn2_scbc")
                bi2_bc = broadcast_row(p7, bi2_row, "bn2_bibc")
                for j in range(NCH):
                    sl = slice(j * CH, (j + 1) * CH)
                    for k in range(CT):
                        tmp = spool.tile([P, CH], fp32, tag="ntmp", bufs=3,
                                         name="ytmp")
                        nc.vector.tensor_mul(tmp[:], u1[k][:, sl],
                                             sc2_bc[:, sl])
                        yk = spool.tile([P, CH], fp32, tag="yout", bufs=3,
                                        name=f"y_{k}_{j}")
                        nc.vector.tensor_add(yk[:], tmp[:], bi2_bc[:, sl])
                        nc.sync.dma_start(yT_d[ts(k, P), sl], yk[:])

    nc.compile()
    return nc


def _get_program():
    global _PROG
    if _PROG is None:
        _PROG = _build()
    return _PROG


def _prep_shared(inputs):
    """Host-side weight folding + pre-tiling; identical for every core."""
    f32 = np.float32
    bf16 = ml_dtypes.bfloat16
    wq = np.asarray(inputs["wq"], f32)      # (H, C, D)
    wk = np.asarray(inputs["wk"], f32)
    wv = np.asarray(inputs["wv"], f32)
    wo = np.asarray(inputs["wo"], f32)      # (C, C)
    bo = np.asarray(inputs["bo"], f32)      # (C,)
    g1 = np.asarray(inputs["ln1_g"], f32)
    b1n = np.asarray(inputs["ln1_b"], f32)
    g2 = np.asarray(inputs["ln2_g"], f32)
    b2n = np.asarray(inputs["ln2_b"], f32)
    w1 = np.asarray(inputs["w1"], f32)      # (C, F)
    b1 = np.asarray(inputs["b1"], f32)      # (F,)
    w2 = np.asarray(inputs["w2"], f32)      # (F, C)
    b2 = np.asarray(inputs["b2"], f32)      # (C,)

    dscale = f32(D) ** f32(-0.5)
    # fold ln1 affine into qkv projections; q also takes 1/sqrt(D)
    wq2 = (wq * g1[None, :, None] * dscale).transpose(1, 0, 2).reshape(C, C)
    wk2 = (wk * g1[None, :, None]).transpose(1, 0, 2).reshape(C, C)
    wv2 = (wv * g1[None, :, None]).transpose(1, 0, 2).reshape(C, C)
    bq = (np.einsum("c,hcd->hd", b1n, wq) * dscale).reshape(C)
    bv = np.einsum("c,hcd->hd", b1n, wv).reshape(C)
    # k-side bias cancels in softmax (constant per row); v bias folds into bo
    bo2 = bo + bv @ wo
    w1f = g2[:, None] * w1
    b1f = b1 + b2n @ w1

    def lhst_tiles(w, n_out):
        # (C_in, n_out*P) -> (n_out, P, C_in//P, P):
        # [o, p, ki, n] = w[ki*P + p, o*P + n]
        ci = w.shape[0]
        return np.ascontiguousarray(
            w.reshape(ci // P, P, n_out, P).transpose(2, 1, 0, 3)
        ).astype(bf16)

    def cols(v, n):  # (n*P,) -> (P, n) with [p, i] = v[i*P + p]
        return np.ascontiguousarray(v.reshape(n, P).T, dtype=f32)

    def row(v):
        return np.ascontiguousarray(v.reshape(1, T), dtype=f32)

    return dict(
        wq=lhst_tiles(wq2, H), wk=lhst_tiles(wk2, H),
        wv=wv2.astype(bf16),
        bq=cols(bq, H), wo=lhst_tiles(wo, CT), bo=cols(bo2, CT),
        w1=lhst_tiles(w1f, FT), b1=cols(b1f, FT),
        w2=lhst_tiles(w2, CT), b2=cols(b2, CT),
        bn1g=row(np.asarray(inputs["bn1_g"], f32)),
        bn1b=row(np.asarray(inputs["bn1_b"], f32)),
        bn2g=row(np.asarray(inputs["bn2_g"], f32)),
        bn2b=row(np.asarray(inputs["bn2_b"], f32)),
    )


def _run(inputs, trace=False):
    from concourse import bass_utils
    nc = _get_program()
    x = np.asarray(inputs["x"], np.float32)
    shared = _prep_shared(inputs)
    in_maps = []
    for b in range(B):
        m = dict(shared)
        m["xT"] = np.ascontiguousarray(x[b].T)
        in_maps.append(m)
    res = bass_utils.run_bass_kernel_spmd(
        nc, in_maps, core_ids=list(range(NCORES)), trace=trace)
    out = np.stack([res.results[b]["yT"].T for b in range(B)]).astype(np.float32)
    return out, res


def kernel(**inputs):
    out, _ = _run(inputs, trace=False)
    return out


# revision 10
# speedup vs baseline: 1.1319x; 1.0897x over previous
"""Trainium2 Bass kernel for nn_Block_50113678410401 (dense transformer block).

Strategy: data-parallel over the batch axis (B=8 -> 8 NeuronCores, one batch
element per core). All on-chip activations live in "layout A": feature axis on
SBUF partitions, token axis (T) on the free dimension, so no on-chip
transposes are needed (host pre-transposes x and post-transposes the output).

Per core:
  LN1 (stats via ones-matmul over partitions), per-head causal attention
  (no-max-sub exp softmax, denominator via ones-matmul, normalization via
  K=1 broadcast matmul + wide reciprocal), output projection + residual,
  BatchNorm over (B,C) with per-512-chunk cross-core AllReduces of
  (sum, sumsq) per T channel (the first chunk's collective overlaps the
  second chunk's matmuls), LN2, FFN (C -> 4C -> relu -> C), residual,
  second BatchNorm.

All big matmuls run in bf16 with fp32 PSUM accumulation; statistics,
softmax, residuals and normalizations are fp32. Weights arrive host-pretiled
so every weight DMA is contiguous per partition.

LayerNorm/projection affine parameters are folded into the weights on the
host: wq' = diag(ln1_g) wq / sqrt(D) (q also carries 1/sqrt(D)), k-side bias
drops out of softmax by shift invariance, v-side bias is folded into the
output-projection bias, ln2 affine is folded into w1/b1.
"""

import numpy as np
import ml_dtypes

B, T, C, H, D = 8, 1024, 1536, 12, 128
F = 4 * C            # 6144
P = 128
CT = C // P          # 12 c-tiles
FT = F // P          # 48 f-tiles
ST = T // P          # 8 s-tiles
CH = 512             # matmul free-dim chunk
NCH = T // CH        # 2 chunks
EPS = 1e-5
NCORES = 8
NBC = B * C          # BatchNorm count over (B, C)

_PROG = None


def _build():
    import concourse.bass as bass
    import concourse.mybir as mybir
    import concourse.tile as tile
    from concourse import bacc
    from concourse.masks import make_upper_triangular

    fp32 = mybir.dt.float32
    bf16 = mybir.dt.bfloat16
    AF = mybir.ActivationFunctionType
    OP = mybir.AluOpType
    ts = bass.ts

    nc = bacc.Bacc("TRN2", target_bir_lowering=False, debug=False,
                   enable_asserts=True, num_devices=NCORES)

    # ---- DRAM I/O (weights host-pretiled for contiguous DMA) ----
    xT_d = nc.dram_tensor("xT", (C, T), fp32, kind="ExternalInput").ap()
    wq_d = nc.dram_tensor("wq", (H, P, CT, P), bf16, kind="ExternalInput").ap()
    wk_d = nc.dram_tensor("wk", (H, P, CT, P), bf16, kind="ExternalInput").ap()
    wv_d = nc.dram_tensor("wv", (C, C), bf16, kind="ExternalInput").ap()
    bq_d = nc.dram_tensor("bq", (P, H), fp32, kind="ExternalInput").ap()
    wo_d = nc.dram_tensor("wo", (CT, P, H, P), bf16, kind="ExternalInput").ap()
    bo_d = nc.dram_tensor("bo", (P, CT), fp32, kind="ExternalInput").ap()
    w1_d = nc.dram_tensor("w1", (FT, P, CT, P), bf16, kind="ExternalInput").ap()
    b1_d = nc.dram_tensor("b1", (P, FT), fp32, kind="ExternalInput").ap()
    w2_d = nc.dram_tensor("w2", (CT, P, FT, P), bf16, kind="ExternalInput").ap()
    b2_d = nc.dram_tensor("b2", (P, CT), fp32, kind="ExternalInput").ap()
    bn1g_d = nc.dram_tensor("bn1g", (1, T), fp32, kind="ExternalInput").ap()
    bn1b_d = nc.dram_tensor("bn1b", (1, T), fp32, kind="ExternalInput").ap()
    bn2g_d = nc.dram_tensor("bn2g", (1, T), fp32, kind="ExternalInput").ap()
    bn2b_d = nc.dram_tensor("bn2b", (1, T), fp32, kind="ExternalInput").ap()
    yT_d = nc.dram_tensor("yT", (C, T), fp32, kind="ExternalOutput").ap()

    with tile.TileContext(nc) as tc:
        with tc.tile_pool(name="const", bufs=1) as cpool, \
             tc.tile_pool(name="scratch", bufs=1) as spool, \
             tc.tile_pool(name="u1p", bufs=1) as u1pool, \
             tc.tile_pool(name="ppw", bufs=6, space="PSUM") as ppw, \
             tc.tile_pool(name="pps", bufs=2, space="PSUM") as pps, \
             tc.tile_pool(name="dram", bufs=1, space="DRAM") as dpool:

            # ---- constants ----
            ones_bf = cpool.tile([P, 1], bf16, name="ones_bf")
            nc.vector.memset(ones_bf[:], 1.0)
            ones1f = cpool.tile([1, P], fp32, name="ones1f")
            nc.vector.memset(ones1f[:], 1.0)
            trimask = cpool.tile([P, P], bf16, name="trimask")
            make_upper_triangular(nc, trimask[:], val=1.0, diag=True)
            eps_col = cpool.tile([P, 1], fp32, name="eps_col")
            nc.vector.memset(eps_col[:], EPS)
            bq_sb = cpool.tile([P, H], fp32, name="bq_sb")
            nc.sync.dma_start(bq_sb[:], bq_d[:])
            bo_sb = cpool.tile([P, CT], fp32, name="bo_sb")
            nc.sync.dma_start(bo_sb[:], bo_d[:])
            b1_sb = cpool.tile([P, FT], fp32, name="b1_sb")
            nc.sync.dma_start(b1_sb[:], b1_d[:])
            b2_sb = cpool.tile([P, CT], fp32, name="b2_sb")
            nc.sync.dma_start(b2_sb[:], b2_d[:])

            # ---- helpers ----
            def bc_mm(row_ap, name):
                """(1, CH) fp32 row -> (P, CH) fp32 PSUM via K=1 matmul."""
                ps = ppw.tile([P, CH], fp32, tag="w", name=f"{name}_ps")
                nc.tensor.matmul(ps[:], ones1f[:], row_ap, start=True,
                                 stop=True)
                return ps

            # Packed stat psum tile: row 0 accumulates sum, row 32 sumsq.
            def stat_tiles(name):
                return [pps.tile([P, CH], fp32, tag="st", bufs=2,
                                 name=f"{name}_{j}") for j in range(NCH)]

            def stats_chunk(src_ap, stp_j, first, last):
                """Ones-matmul partial sums of src chunk (fp32 (P,CH)) and its
                square into packed stat rows."""
                cbf = spool.tile([P, CH], bf16, tag="cast_bf", bufs=2,
                                 name="cbf")
                nc.vector.tensor_copy(cbf[:], src_ap)
                csq = spool.tile([P, CH], bf16, tag="cast_sq", bufs=2,
                                 name="csq")
                nc.scalar.square(csq[:], src_ap)
                nc.tensor.matmul(stp_j[0:1, :], ones_bf[:], cbf[:],
                                 start=first, stop=last)
                nc.tensor.matmul(stp_j[32:33, :], ones_bf[:], csq[:],
                                 start=first, stop=last)

            def allreduce_chunk(pool, stp_j, name):
                """AllReduce-add this chunk's packed (sum, sumsq) across
                cores. Returns the (1, 2*CH) global row."""
                loc = pool.tile([1, 2 * CH], fp32, tag="arloc", bufs=1,
                                name=f"{name}_loc")
                nc.scalar.copy(loc[:, 0:CH], stp_j[0:1, :])
                nc.scalar.copy(loc[:, CH:2 * CH], stp_j[32:33, :])
                cin = dpool.tile([1, 2 * CH], fp32, name=f"{name}_cin")
                cout = dpool.tile([1, 2 * CH], fp32, name=f"{name}_cout")
                nc.sync.dma_start(cin[:], loc[:])
                nc.gpsimd.collective_compute(
                    "AllReduce", mybir.AluOpType.add,
                    replica_groups=[list(range(NCORES))],
                    ins=[cin.opt()], outs=[cout.opt()],
                )
                glob = pool.tile([1, 2 * CH], fp32, tag="arglob", bufs=2,
                                 name=f"{name}_glob")
                nc.sync.dma_start(glob[:], cout[:])
                return glob

            def norm_params_chunk(pool, s1_ap, s2_ap, count, name,
                                  g_bc_sl=None, b_bc_sl=None):
                """Per-chunk normalization params, broadcast to (P, CH):
                scale = rstd (* g), bias = -mean*scale (+ b). The reciprocal
                runs wide (128 lanes) on the broadcast std."""
                m = pool.tile([1, CH], fp32, tag="rm", bufs=2,
                              name=f"{name}_m")
                nc.vector.tensor_scalar_mul(m[:], s1_ap, 1.0 / count)
                v = pool.tile([1, CH], fp32, tag="rv", bufs=2,
                              name=f"{name}_v")
                nc.vector.tensor_scalar_mul(v[:], s2_ap, 1.0 / count)
                msq = pool.tile([1, CH], fp32, tag="rq", bufs=2,
                                name=f"{name}_msq")
                nc.vector.tensor_mul(msq[:], m[:], m[:])
                nc.vector.tensor_sub(v[:], v[:], msq[:])
                m_ps = bc_mm(m[:], f"{name}_mbc")
                v_ps = bc_mm(v[:], f"{name}_vbc")
                st = spool.tile([P, CH], fp32, tag="nst", bufs=1,
                                name=f"{name}_st")
                nc.scalar.activation(st[:], v_ps[:], AF.Sqrt,
                                     bias=eps_col[:, 0:1], scale=1.0)
                scale = pool.tile([P, CH], fp32, tag="nsc", bufs=2,
                                  name=f"{name}_scale")
                if g_bc_sl is not None:
                    rc = spool.tile([P, CH], fp32, tag="nrc", bufs=1,
                                    name=f"{name}_rc")
                    nc.vector.reciprocal(rc[:], st[:])
                    nc.vector.tensor_mul(scale[:], rc[:], g_bc_sl)
                else:
                    nc.vector.reciprocal(scale[:], st[:])
                mt = spool.tile([P, CH], fp32, tag="nmt", bufs=1,
                                name=f"{name}_mt")
                nc.vector.tensor_mul(mt[:], m_ps[:], scale[:])
                bias = pool.tile([P, CH], fp32, tag="nbi", bufs=2,
                                 name=f"{name}_bias")
                if b_bc_sl is not None:
                    nc.vector.tensor_sub(bias[:], b_bc_sl, mt[:])
                else:
                    nc.vector.tensor_scalar_mul(bias[:], mt[:], -1.0)
                return scale, bias

            def broadcast_row_full(pool, row, name, tag="bc", bufs=2):
                """(1, T) fp32 -> (P, T) fp32 SBUF via K=1 matmuls."""
                bc = pool.tile([P, T], fp32, tag=tag, bufs=bufs, name=name)
                for j in range(NCH):
                    sl = slice(j * CH, (j + 1) * CH)
                    ps = bc_mm(row[:, sl], f"{name}{j}")
                    nc.scalar.copy(bc[:, sl], ps[:])
                return bc

            def affine_chunk(dst_ap, src_ap, sc_ap, bi_ap):
                """dst = src * sc + bi on one (P, CH) chunk."""
                tmp = spool.tile([P, CH], fp32, tag="ntmp", bufs=2,
                                 name="ntmp")
                nc.vector.tensor_mul(tmp[:], src_ap, sc_ap)
                nc.vector.tensor_add(dst_ap, tmp[:], bi_ap)

            u1 = []     # created at phase 4 (first use)
            o_nrm = []  # created at phase 3

            with tc.tile_pool(name="onrm", bufs=1) as opool:
                with tc.tile_pool(name="hT", bufs=1) as hpool:
                    hT = [hpool.tile([P, T], bf16, tag=f"h{k}", name=f"hT_{k}")
                          for k in range(CT)]
                    # ================= Phase 1: LN1 =================
                    with tc.tile_pool(name="p1", bufs=1) as p1:
                        stp = stat_tiles("ln1")
                        x_sb = []
                        for k in range(CT):
                            xk = p1.tile([P, T], fp32, tag=f"x{k}", name=f"x_{k}")
                            nc.sync.dma_start(xk[:], xT_d[ts(k, P), :])
                            x_sb.append(xk)
                            for j in range(NCH):
                                sl = slice(j * CH, (j + 1) * CH)
                                stats_chunk(xk[:, sl], stp[j], k == 0,
                                            k == CT - 1)
                        for j in range(NCH):
                            sl = slice(j * CH, (j + 1) * CH)
                            sc, bi = norm_params_chunk(
                                p1, stp[j][0:1, :], stp[j][32:33, :], C,
                                f"ln1_{j}")
                            for k in range(CT):
                                affine_chunk(hT[k][:, sl], x_sb[k][:, sl],
                                             sc[:], bi[:])

                    # ================= Phase 2: V for all heads ============
                    with tc.tile_pool(name="vall", bufs=1) as vpool:
                        Vall = [vpool.tile([P, C], bf16, tag=f"v{s}",
                                           name=f"V_{s}") for s in range(ST)]
                        with tc.tile_pool(name="wv", bufs=1) as wvpool:
                            wv_sb = []
                            for k in range(CT):
                                wvk = wvpool.tile([P, C], bf16, tag=f"wv{k}",
                                                  name=f"wv_{k}")
                                nc.sync.dma_start(wvk[:], wv_d[ts(k, P), :])
                                wv_sb.append(wvk)
                            for s in range(ST):
                                for n in range(C // CH):
                                    vps = ppw.tile([P, CH], fp32, tag="w",
                                                   name=f"v_ps_{s}_{n}")
                                    for k in range(CT):
                                        nc.tensor.matmul(
                                            vps[:], hT[k][:, ts(s, P)],
                                            wv_sb[k][:, ts(n, CH)],
                                            start=(k == 0), stop=(k == CT - 1))
                                    nc.scalar.copy(Vall[s][:, ts(n, CH)], vps[:])

                        # ============ Phase 3: per-head attention ==========
                        with tc.tile_pool(name="p3", bufs=1) as p3:
                            for h in range(H):
                                o_nrm.append(opool.tile(
                                    [P, T], bf16, tag=f"o{h}", name=f"on_{h}"))
                                wqh = p3.tile([P, CT, P], bf16, tag="wqh",
                                              bufs=2, name=f"wqh_{h}")
                                nc.sync.dma_start(wqh[:], wq_d[h])
                                wkh = p3.tile([P, CT, P], bf16, tag="wkh",
                                              bufs=2, name=f"wkh_{h}")
                                nc.sync.dma_start(wkh[:], wk_d[h])
                                qT = p3.tile([P, T], bf16, tag="qT", bufs=2,
                                             name=f"qT_{h}")
                                kT = p3.tile([P, T], bf16, tag="kT", bufs=2,
                                             name=f"kT_{h}")
                                for j in range(NCH):
                                    sl = slice(j * CH, (j + 1) * CH)
                                    qps = ppw.tile([P, CH], fp32, tag="w",
                                                   name=f"q_ps_{h}_{j}")
                                    for k in range(CT):
                                        nc.tensor.matmul(qps[:], wqh[:, k, :],
                                                         hT[k][:, sl],
                                                         start=(k == 0),
                                                         stop=(k == CT - 1))
                                    nc.scalar.activation(qT[:, sl], qps[:],
                                                         AF.Identity,
                                                         bias=bq_sb[:, h:h + 1],
                                                         scale=1.0)
                                    kps = ppw.tile([P, CH], fp32, tag="w",
                                                   name=f"k_ps_{h}_{j}")
                                    for k in range(CT):
                                        nc.tensor.matmul(kps[:], wkh[:, k, :],
                                                         hT[k][:, sl],
                                                         start=(k == 0),
                                                         stop=(k == CT - 1))
                                    nc.scalar.copy(kT[:, sl], kps[:])
                                # scores + exp (causal: s-tile covers t >= s*P)
                                aT = []
                                for s in range(ST):
                                    at = p3.tile([P, T], bf16, tag=f"a{s}",
                                                 bufs=1, name=f"aT_{h}_{s}")
                                    aT.append(at)
                                    for j in range(NCH):
                                        lo = max(j * CH, s * P)
                                        hi = (j + 1) * CH
                                        if lo >= hi:
                                            continue
                                        sps = ppw.tile([P, CH], fp32, tag="w",
                                                       name=f"s_ps_{h}_{s}_{j}")
                                        nc.tensor.matmul(sps[:, :hi - lo],
                                                         kT[:, ts(s, P)],
                                                         qT[:, lo:hi],
                                                         start=True, stop=True)
                                        nc.scalar.activation(at[:, lo:hi],
                                                             sps[:, :hi - lo],
                                                             AF.Exp)
                                    nc.vector.tensor_mul(at[:, ts(s, P)],
                                                         at[:, ts(s, P)],
                                                         trimask[:])
                                # denominators: packed psum, row 0 (j=0)
                                # and row 32 (j=1)
                                den_ps = pps.tile([P, CH], fp32, tag="st",
                                                  bufs=2, name=f"dn_{h}")
                                for j in range(NCH):
                                    r0 = 32 * j
                                    smax = min(ST, 4 * (j + 1))
                                    for s in range(smax):
                                        lo = max(0, s * P - j * CH)
                                        nc.tensor.matmul(
                                            den_ps[r0:r0 + 1, lo:CH],
                                            ones_bf[:],
                                            aT[s][:, j * CH + lo:(j + 1) * CH],
                                            start=(s == 0), stop=(s == smax - 1))
                                # r_bc = 1/den broadcast: copy row, K=1 mm,
                                # then wide reciprocal straight off PSUM
                                r_bc = p3.tile([P, T], fp32, tag="rbc", bufs=2,
                                               name=f"rbc_{h}")
                                for j in range(NCH):
                                    dj = p3.tile([1, CH], fp32, tag="den",
                                                 bufs=2, name=f"den_{h}_{j}")
                                    nc.scalar.copy(
                                        dj[:], den_ps[32 * j:32 * j + 1, :])
                                    dps = bc_mm(dj[:], f"dbc_{h}_{j}")
                                    nc.vector.reciprocal(
                                        r_bc[:, j * CH:(j + 1) * CH], dps[:])
                                # attention @ V, then normalize
                                for j in range(NCH):
                                    smax = min(ST, 4 * (j + 1))
                                    ops_ = ppw.tile([P, CH], fp32, tag="w",
                                                    name=f"o_ps_{h}_{j}")
                                    for s in range(smax):
                                        lo = max(0, s * P - j * CH)
                                        nc.tensor.matmul(
                                            ops_[:, lo:CH],
                                            Vall[s][:, ts(h, P)],
                                            aT[s][:, j * CH + lo:(j + 1) * CH],
                                            start=(s == 0), stop=(s == smax - 1))
                                    sl = slice(j * CH, (j + 1) * CH)
                                    nc.vector.tensor_mul(o_nrm[h][:, sl],
                                                         ops_[:], r_bc[:, sl])

                # hT closed; Phase 4: out-proj + residual + BN1 stats
                # (j-outer so chunk 0's AllReduce overlaps chunk 1's matmuls)
                stp_bn1 = stat_tiles("bn1")
                glob_bn1 = [None, None]
                with tc.tile_pool(name="p4", bufs=1) as p4:
                    wok_sb = []
                    x2_sb = []
                    for k in range(CT):
                        wok = p4.tile([P, H, P], bf16, tag=f"wok{k}",
                                      name=f"wok_{k}")
                        nc.sync.dma_start(wok[:], wo_d[k])
                        wok_sb.append(wok)
                        x2k = p4.tile([P, T], fp32, tag=f"x2{k}",
                                      name=f"x2_{k}")
                        nc.sync.dma_start(x2k[:], xT_d[ts(k, P), :])
                        x2_sb.append(x2k)
                        u1.append(u1pool.tile([P, T], fp32, tag=f"u{k}",
                                              name=f"u1_{k}"))
                    for j in range(NCH):
                        sl = slice(j * CH, (j + 1) * CH)
                        for k in range(CT):
                            saps = ppw.tile([P, CH], fp32, tag="w",
                                            name=f"sa_ps_{k}_{j}")
                            for hh in range(H):
                                nc.tensor.matmul(saps[:], wok_sb[k][:, hh, :],
                                                 o_nrm[hh][:, sl],
                                                 start=(hh == 0),
                                                 stop=(hh == H - 1))
                            nc.vector.scalar_tensor_tensor(
                                out=u1[k][:, sl], in0=saps[:],
                                scalar=bo_sb[:, k:k + 1], in1=x2_sb[k][:, sl],
                                op0=OP.add, op1=OP.add)
                            stats_chunk(u1[k][:, sl], stp_bn1[j],
                                        k == 0, k == CT - 1)
                        glob_bn1[j] = allreduce_chunk(u1pool, stp_bn1[j],
                                                      f"bn1_{j}")

            # ================= Phase 5: BN1 + LN2 =================
            with tc.tile_pool(name="h2T", bufs=1) as h2pool:
                h2T = [h2pool.tile([P, T], bf16, tag=f"h2{k}", name=f"h2_{k}")
                       for k in range(CT)]
                stp_bn2 = stat_tiles("bn2")
                glob_bn2 = [None, None]
                with tc.tile_pool(name="p5", bufs=1) as p5:
                    bn1g_sb = p5.tile([1, T], fp32, name="bn1g_sb")
                    nc.sync.dma_start(bn1g_sb[:], bn1g_d[:])
                    bn1b_sb = p5.tile([1, T], fp32, name="bn1b_sb")
                    nc.sync.dma_start(bn1b_sb[:], bn1b_d[:])
                    g1bc = broadcast_row_full(p5, bn1g_sb, "bn1g_bc", tag="gbc")
                    b1bc = broadcast_row_full(p5, bn1b_sb, "bn1b_bc", tag="bbc")
                    stp_ln2 = stat_tiles("ln2")
                    for j in range(NCH):
                        sl = slice(j * CH, (j + 1) * CH)
                        sc, bi = norm_params_chunk(
                            p5, glob_bn1[j][:, 0:CH], glob_bn1[j][:, CH:2 * CH],
                            NBC, f"bn1_{j}", g_bc_sl=g1bc[:, sl],
                            b_bc_sl=b1bc[:, sl])
                        for k in range(CT):
                            affine_chunk(u1[k][:, sl], u1[k][:, sl],
                                         sc[:], bi[:])
                            stats_chunk(u1[k][:, sl], stp_ln2[j],
                                        k == 0, k == CT - 1)
                        sc2, bi2 = norm_params_chunk(
                            p5, stp_ln2[j][0:1, :], stp_ln2[j][32:33, :], C,
                            f"ln2_{j}")
                        for k in range(CT):
                            affine_chunk(h2T[k][:, sl], u1[k][:, sl],
                                         sc2[:], bi2[:])

                # ================= Phase 6: FFN =================
                with tc.tile_pool(name="p6", bufs=1) as p6:
                    for j in range(NCH):
                        sl = slice(j * CH, (j + 1) * CH)
                        z = []
                        for f in range(FT):
                            w1f = p6.tile([P, CT, P], bf16, tag="w1f", bufs=2,
                                          name=f"w1f_{j}_{f}")
                            nc.sync.dma_start(w1f[:], w1_d[f])
                            zps = ppw.tile([P, CH], fp32, tag="w",
                                           name=f"z_ps_{j}_{f}")
                            for k in range(CT):
                                nc.tensor.matmul(zps[:], w1f[:, k, :],
                                                 h2T[k][:, sl],
                                                 start=(k == 0),
                                                 stop=(k == CT - 1))
                            zf = p6.tile([P, CH], bf16, tag=f"z{f}",
                                         name=f"z_{j}_{f}")
                            nc.scalar.activation(zf[:], zps[:], AF.Relu,
                                                 bias=b1_sb[:, f:f + 1],
                                                 scale=1.0)
                            z.append(zf)
                        for k in range(CT):
                            w2k = p6.tile([P, FT, P], bf16, tag="w2k", bufs=2,
                                          name=f"w2k_{j}_{k}")
                            nc.sync.dma_start(w2k[:], w2_d[k])
                            yps = ppw.tile([P, CH], fp32, tag="w",
                                           name=f"y_ps_{j}_{k}")
                            for f in range(FT):
                                nc.tensor.matmul(yps[:], w2k[:, f, :], z[f][:],
                                                 start=(f == 0),
                                                 stop=(f == FT - 1))
                            nc.vector.scalar_tensor_tensor(
                                out=u1[k][:, sl], in0=yps[:],
                                scalar=b2_sb[:, k:k + 1], in1=u1[k][:, sl],
                                op0=OP.add, op1=OP.add)
                            stats_chunk(u1[k][:, sl], stp_bn2[j],
                                        k == 0, k == CT - 1)
                        glob_bn2[j] = allreduce_chunk(u1pool, stp_bn2[j],
                                                      f"bn2_{j}")

            # ================= Phase 7: BN2 + output =================
            with tc.tile_pool(name="p7", bufs=1) as p7:
                bn2g_sb = p7.tile([1, T], fp32, name="bn2g_sb")
                nc.sync.dma_start(bn2g_sb[:], bn2g_d[:])
                bn2b_sb = p7.tile([1, T], fp32, name="bn2b_sb")
                nc.sync.dma_start(bn2b_sb[:], bn2b_d[:])
                g2bc = broadcast_row_full(p7, bn2g_sb, "bn2g_bc", tag="gbc")
                b2bc = broadcast_row_full(p7, bn2b_sb, "bn2b_bc", tag="bbc")
                for j in range(NCH):
                    sl = slice(j * CH, (j + 1) * CH)
                    sc, bi = norm_params_chunk(
                        p7, glob_bn2[j][:, 0:CH], glob_bn2[j][:, CH:2 * CH],
                        NBC, f"bn2_{j}", g_bc_sl=g2bc[:, sl],
                        b_bc_sl=b2bc[:, sl])
                    for k in range(CT):
                        tmp = spool.tile([P, CH], fp32, tag="ntmp", bufs=2,
                                         name="ytmp")
                        nc.vector.tensor_mul(tmp[:], u1[k][:, sl], sc[:])
                        yk = spool.tile([P, CH], fp32, tag="yout", bufs=2,
                                        name=f"y_{k}_{j}")
                        nc.vector.tensor_add(yk[:], tmp[:], bi[:])
                        nc.sync.dma_start(yT_d[ts(k, P), sl], yk[:])

    nc.compile()
    return nc


def _get_program():
    global _PROG
    if _PROG is None:
        _PROG = _build()
    return _PROG


def _prep_shared(inputs):
    """Host-side weight folding + pre-tiling; identical for every core."""
    f32 = np.float32
    bf16 = ml_dtypes.bfloat16
    wq = np.asarray(inputs["wq"], f32)      # (H, C, D)
    wk = np.asarray(inputs["wk"], f32)
    wv = np.asarray(inputs["wv"], f32)
    wo = np.asarray(inputs["wo"], f32)      # (C, C)
    bo = np.asarray(inputs["bo"], f32)      # (C,)
    g1 = np.asarray(inputs["ln1_g"], f32)
    b1n = np.asarray(inputs["ln1_b"], f32)
    g2 = np.asarray(inputs["ln2_g"], f32)
    b2n = np.asarray(inputs["ln2_b"], f32)
    w1 = np.asarray(inputs["w1"], f32)      # (C, F)
    b1 = np.asarray(inputs["b1"], f32)      # (F,)
    w2 = np.asarray(inputs["w2"], f32)      # (F, C)
    b2 = np.asarray(inputs["b2"], f32)      # (C,)

    dscale = f32(D) ** f32(-0.5)
    # fold ln1 affine into qkv projections; q also takes 1/sqrt(D)
    wq2 = (wq * g1[None, :, None] * dscale).transpose(1, 0, 2).reshape(C, C)
    wk2 = (wk * g1[None, :, None]).transpose(1, 0, 2).reshape(C, C)
    wv2 = (wv * g1[None, :, None]).transpose(1, 0, 2).reshape(C, C)
    bq = (np.einsum("c,hcd->hd", b1n, wq) * dscale).reshape(C)
    bv = np.einsum("c,hcd->hd", b1n, wv).reshape(C)
    # k-side bias cancels in softmax (constant per row); v bias folds into bo
    bo2 = bo + bv @ wo
    w1f = g2[:, None] * w1
    b1f = b1 + b2n @ w1

    def lhst_tiles(w, n_out):
        # (C_in, n_out*P) -> (n_out, P, C_in//P, P):
        # [o, p, ki, n] = w[ki*P + p, o*P + n]
        ci = w.shape[0]
        return np.ascontiguousarray(
            w.reshape(ci // P, P, n_out, P).transpose(2, 1, 0, 3)
        ).astype(bf16)

    def cols(v, n):  # (n*P,) -> (P, n) with [p, i] = v[i*P + p]
        return np.ascontiguousarray(v.reshape(n, P).T, dtype=f32)

    def row(v):
        return np.ascontiguousarray(v.reshape(1, T), dtype=f32)

    return dict(
        wq=lhst_tiles(wq2, H), wk=lhst_tiles(wk2, H),
        wv=wv2.astype(bf16),
        bq=cols(bq, H), wo=lhst_tiles(wo, CT), bo=cols(bo2, CT),
        w1=lhst_tiles(w1f, FT), b1=cols(b1f, FT),
        w2=lhst_tiles(w2, CT), b2=cols(b2, CT),
        bn1g=row(np.asarray(inputs["bn1_g"], f32)),
        bn1b=row(np.asarray(inputs["bn1_b"], f32)),
        bn2g=row(np.asarray(inputs["bn2_g"], f32)),
        bn2b=row(np.asarray(inputs["bn2_b"], f32)),
    )


def _run(inputs, trace=False):
    from concourse import bass_utils
    nc = _get_program()
    x = np.asarray(inputs["x"], np.float32)
    shared = _prep_shared(inputs)
    in_maps = []
    for b in range(B):
        m = dict(shared)
        m["xT"] = np.ascontiguousarray(x[b].T)
        in_maps.append(m)
    res = bass_utils.run_bass_kernel_spmd(
        nc, in_maps, core_ids=list(range(NCORES)), trace=trace)
    out = np.stack([res.results[b]["yT"].T for b in range(B)]).astype(np.float32)
    return out, res


def kernel(**inputs):
    out, _ = _run(inputs, trace=False)
    return out


# revision 13
# speedup vs baseline: 1.1644x; 1.0287x over previous
"""Trainium2 Bass kernel for nn_Block_50113678410401 (dense transformer block).

Strategy: data-parallel over the batch axis (B=8 -> 8 NeuronCores, one batch
element per core). All on-chip activations live in "layout A": feature axis on
SBUF partitions, token axis (T) on the free dimension, so no on-chip
transposes are needed (host pre-transposes x and post-transposes the output).

Per core:
  LN1 (stats via ones-matmul over partitions), per-head causal attention
  (no-max-sub exp softmax, denominator via ones-matmul, normalization via
  K=1 broadcast matmul + wide reciprocal), output projection + residual,
  BatchNorm over (B,C) with per-512-chunk cross-core AllReduces of
  (sum, sumsq) per T channel (the first chunk's collective overlaps the
  second chunk's matmuls), LN2, FFN (C -> 4C -> relu -> C), residual,
  second BatchNorm.

All big matmuls run in bf16 with fp32 PSUM accumulation; statistics,
softmax, residuals and normalizations are fp32. Weights arrive host-pretiled
so every weight DMA is contiguous per partition.

LayerNorm/projection affine parameters are folded into the weights on the
host: wq' = diag(ln1_g) wq / sqrt(D) (q also carries 1/sqrt(D)), k-side bias
drops out of softmax by shift invariance, v-side bias is folded into the
output-projection bias, ln2 affine is folded into w1/b1.
"""

import numpy as np
import ml_dtypes

B, T, C, H, D = 8, 1024, 1536, 12, 128
F = 4 * C            # 6144
P = 128
CT = C // P          # 12 c-tiles
FT = F // P          # 48 f-tiles
ST = T // P          # 8 s-tiles
CH = 512             # matmul free-dim chunk
NCH = T // CH        # 2 chunks
EPS = 1e-5
NCORES = 8
NBC = B * C          # BatchNorm count over (B, C)

_PROG = None


def _build():
    import concourse.bass as bass
    import concourse.mybir as mybir
    import concourse.tile as tile
    from concourse import bacc
    from concourse.masks import make_upper_triangular

    fp32 = mybir.dt.float32
    bf16 = mybir.dt.bfloat16
    AF = mybir.ActivationFunctionType
    OP = mybir.AluOpType
    ts = bass.ts

    nc = bacc.Bacc("TRN2", target_bir_lowering=False, debug=False,
                   enable_asserts=True, num_devices=NCORES)

    # ---- DRAM I/O (weights host-pretiled for contiguous DMA) ----
    xT_d = nc.dram_tensor("xT", (C, T), fp32, kind="ExternalInput").ap()
    xbf_d = nc.dram_tensor("xbf", (C, T), bf16, kind="ExternalInput").ap()
    wq_d = nc.dram_tensor("wq", (H, P, CT, P), bf16, kind="ExternalInput").ap()
    wk_d = nc.dram_tensor("wk", (H, P, CT, P), bf16, kind="ExternalInput").ap()
    wv_d = nc.dram_tensor("wv", (C, C), bf16, kind="ExternalInput").ap()
    bq_d = nc.dram_tensor("bq", (P, H), fp32, kind="ExternalInput").ap()
    wo_d = nc.dram_tensor("wo", (CT, P, H, P), bf16, kind="ExternalInput").ap()
    bo_d = nc.dram_tensor("bo", (P, CT), fp32, kind="ExternalInput").ap()
    w1_d = nc.dram_tensor("w1", (FT, P, CT, P), bf16, kind="ExternalInput").ap()
    b1_d = nc.dram_tensor("b1", (P, FT), fp32, kind="ExternalInput").ap()
    w2_d = nc.dram_tensor("w2", (CT, P, FT, P), bf16, kind="ExternalInput").ap()
    b2_d = nc.dram_tensor("b2", (P, CT), fp32, kind="ExternalInput").ap()
    bn1g_d = nc.dram_tensor("bn1g", (1, T), fp32, kind="ExternalInput").ap()
    bn1b_d = nc.dram_tensor("bn1b", (1, T), fp32, kind="ExternalInput").ap()
    bn2g_d = nc.dram_tensor("bn2g", (1, T), fp32, kind="ExternalInput").ap()
    bn2b_d = nc.dram_tensor("bn2b", (1, T), fp32, kind="ExternalInput").ap()
    yT_d = nc.dram_tensor("yT", (C, T), fp32, kind="ExternalOutput").ap()

    with tile.TileContext(nc) as tc:
        with tc.tile_pool(name="const", bufs=1) as cpool, \
             tc.tile_pool(name="scratch", bufs=1) as spool, \
             tc.tile_pool(name="u1p", bufs=1) as u1pool, \
             tc.tile_pool(name="ppw", bufs=6, space="PSUM") as ppw, \
             tc.tile_pool(name="pps", bufs=2, space="PSUM") as pps, \
             tc.tile_pool(name="dram", bufs=1, space="DRAM") as dpool:

            # ---- constants ----
            ones_bf = cpool.tile([P, 1], bf16, name="ones_bf")
            nc.vector.memset(ones_bf[:], 1.0)
            ones1f = cpool.tile([1, P], fp32, name="ones1f")
            nc.vector.memset(ones1f[:], 1.0)
            trimask = cpool.tile([P, P], bf16, name="trimask")
            make_upper_triangular(nc, trimask[:], val=1.0, diag=True)
            eps_col = cpool.tile([P, 1], fp32, name="eps_col")
            nc.vector.memset(eps_col[:], EPS)
            bq_sb = cpool.tile([P, H], fp32, name="bq_sb")
            nc.sync.dma_start(bq_sb[:], bq_d[:])
            bo_sb = cpool.tile([P, CT], fp32, name="bo_sb")
            nc.sync.dma_start(bo_sb[:], bo_d[:])
            b1_sb = cpool.tile([P, FT], fp32, name="b1_sb")
            nc.sync.dma_start(b1_sb[:], b1_d[:])
            b2_sb = cpool.tile([P, CT], fp32, name="b2_sb")
            nc.sync.dma_start(b2_sb[:], b2_d[:])

            # ---- helpers ----
            def bc_mm(row_ap, name):
                """(1, CH) fp32 row -> (P, CH) fp32 PSUM via K=1 matmul."""
                ps = ppw.tile([P, CH], fp32, tag="w", name=f"{name}_ps")
                nc.tensor.matmul(ps[:], ones1f[:], row_ap, start=True,
                                 stop=True)
                return ps

            # Packed stat psum tile: row 0 accumulates sum, row 32 sumsq.
            def stat_tiles(name):
                return [pps.tile([P, CH], fp32, tag="st", bufs=2,
                                 name=f"{name}_{j}") for j in range(NCH)]

            def stats_chunk(src_ap, stp_j, first, last, is_bf16=False):
                """Ones-matmul partial sums of src chunk ((P,CH)) and its
                square into packed stat rows."""
                if is_bf16:
                    cbf = src_ap
                else:
                    cbf_t = spool.tile([P, CH], bf16, tag="cast_bf", bufs=2,
                                       name="cbf")
                    nc.vector.tensor_copy(cbf_t[:], src_ap)
                    cbf = cbf_t[:]
                csq = spool.tile([P, CH], bf16, tag="cast_sq", bufs=2,
                                 name="csq")
                nc.scalar.square(csq[:], src_ap)
                nc.tensor.matmul(stp_j[0:1, :], ones_bf[:], cbf,
                                 start=first, stop=last)
                nc.tensor.matmul(stp_j[32:33, :], ones_bf[:], csq[:],
                                 start=first, stop=last)

            def allreduce_chunk(pool, stp_j, name):
                """AllReduce-add this chunk's packed (sum, sumsq) across
                cores. Returns the (1, 2*CH) global row."""
                loc = pool.tile([1, 2 * CH], fp32, tag="arloc", bufs=2,
                                name=f"{name}_loc")
                nc.scalar.copy(loc[:, 0:CH], stp_j[0:1, :])
                nc.scalar.copy(loc[:, CH:2 * CH], stp_j[32:33, :])
                cin = dpool.tile([1, 2 * CH], fp32, name=f"{name}_cin")
                cout = dpool.tile([1, 2 * CH], fp32, name=f"{name}_cout")
                nc.sync.dma_start(cin[:], loc[:])
                nc.gpsimd.collective_compute(
                    "AllReduce", mybir.AluOpType.add,
                    replica_groups=[list(range(NCORES))],
                    ins=[cin.opt()], outs=[cout.opt()],
                )
                glob = pool.tile([1, 2 * CH], fp32, tag="arglob", bufs=2,
                                 name=f"{name}_glob")
                nc.sync.dma_start(glob[:], cout[:])
                return loc, glob

            def norm_params_chunk(pool, s1_ap, s2_ap, count, name,
                                  g_bc_sl=None, b_bc_sl=None,
                                  sc_tag="nsc", bi_tag="nbi"):
                """Per-chunk normalization params, broadcast to (P, CH):
                scale = rstd (* g), bias = -mean*scale (+ b). The reciprocal
                runs wide (128 lanes) on the broadcast std."""
                m = pool.tile([1, CH], fp32, tag="rm", bufs=2,
                              name=f"{name}_m")
                nc.vector.tensor_scalar_mul(m[:], s1_ap, 1.0 / count)
                v = pool.tile([1, CH], fp32, tag="rv", bufs=2,
                              name=f"{name}_v")
                nc.vector.tensor_scalar_mul(v[:], s2_ap, 1.0 / count)
                msq = pool.tile([1, CH], fp32, tag="rq", bufs=2,
                                name=f"{name}_msq")
                nc.vector.tensor_mul(msq[:], m[:], m[:])
                nc.vector.tensor_sub(v[:], v[:], msq[:])
                m_ps = bc_mm(m[:], f"{name}_mbc")
                v_ps = bc_mm(v[:], f"{name}_vbc")
                st = spool.tile([P, CH], fp32, tag="nst", bufs=1,
                                name=f"{name}_st")
                nc.scalar.activation(st[:], v_ps[:], AF.Sqrt,
                                     bias=eps_col[:, 0:1], scale=1.0)
                scale = pool.tile([P, CH], fp32, tag=sc_tag, bufs=2,
                                  name=f"{name}_scale")
                if g_bc_sl is not None:
                    rc = spool.tile([P, CH], fp32, tag="nrc", bufs=1,
                                    name=f"{name}_rc")
                    nc.vector.reciprocal_approx_fast(rc[:], st[:])
                    nc.vector.tensor_mul(scale[:], rc[:], g_bc_sl)
                else:
                    nc.vector.reciprocal_approx_fast(scale[:], st[:])
                mt = spool.tile([P, CH], fp32, tag="nmt", bufs=1,
                                name=f"{name}_mt")
                nc.vector.tensor_mul(mt[:], m_ps[:], scale[:])
                bias = pool.tile([P, CH], fp32, tag=bi_tag, bufs=2,
                                 name=f"{name}_bias")
                if b_bc_sl is not None:
                    nc.vector.tensor_sub(bias[:], b_bc_sl, mt[:])
                else:
                    nc.vector.tensor_scalar_mul(bias[:], mt[:], -1.0)
                return scale, bias

            def broadcast_row_full(pool, row, name, tag="bc", bufs=2):
                """(1, T) fp32 -> (P, T) fp32 SBUF via K=1 matmuls."""
                bc = pool.tile([P, T], fp32, tag=tag, bufs=bufs, name=name)
                for j in range(NCH):
                    sl = slice(j * CH, (j + 1) * CH)
                    ps = bc_mm(row[:, sl], f"{name}{j}")
                    nc.scalar.copy(bc[:, sl], ps[:])
                return bc

            def affine_chunk(dst_ap, src_ap, sc_ap, bi_ap):
                """dst = src * sc + bi on one (P, CH) chunk."""
                tmp = spool.tile([P, CH], fp32, tag="ntmp", bufs=2,
                                 name="ntmp")
                nc.vector.tensor_mul(tmp[:], src_ap, sc_ap)
                nc.vector.tensor_add(dst_ap, tmp[:], bi_ap)

            u1 = []     # created at phase 4 (first use)
            o_nrm = []  # created at phase 3

            with tc.tile_pool(name="onrm", bufs=1) as opool:
                with tc.tile_pool(name="hT", bufs=1) as hpool:
                    hT = [hpool.tile([P, T], bf16, tag=f"h{k}", name=f"hT_{k}")
                          for k in range(CT)]
                    # ================= Phase 1: LN1 =================
                    with tc.tile_pool(name="p1", bufs=1) as p1:
                        stp = stat_tiles("ln1")
                        x_sb = []
                        for k in range(CT):
                            xk = p1.tile([P, T], bf16, tag=f"x{k}", name=f"x_{k}")
                            nc.sync.dma_start(xk[:], xbf_d[ts(k, P), :])
                            x_sb.append(xk)
                            for j in range(NCH):
                                sl = slice(j * CH, (j + 1) * CH)
                                stats_chunk(xk[:, sl], stp[j], k == 0,
                                            k == CT - 1, is_bf16=True)
                        for j in range(NCH):
                            sl = slice(j * CH, (j + 1) * CH)
                            sc, bi = norm_params_chunk(
                                p1, stp[j][0:1, :], stp[j][32:33, :], C,
                                f"ln1_{j}")
                            for k in range(CT):
                                affine_chunk(hT[k][:, sl], x_sb[k][:, sl],
                                             sc[:], bi[:])

                    # ================= Phase 2: V for all heads ============
                    with tc.tile_pool(name="vall", bufs=1) as vpool:
                        Vall = [vpool.tile([P, C], bf16, tag=f"v{s}",
                                           name=f"V_{s}") for s in range(ST)]
                        with tc.tile_pool(name="wv", bufs=1) as wvpool:
                            wv_sb = []
                            for k in range(CT):
                                wvk = wvpool.tile([P, C], bf16, tag=f"wv{k}",
                                                  name=f"wv_{k}")
                                nc.sync.dma_start(wvk[:], wv_d[ts(k, P), :])
                                wv_sb.append(wvk)
                            for s in range(ST):
                                for n in range(C // CH):
                                    vps = ppw.tile([P, CH], fp32, tag="w",
                                                   name=f"v_ps_{s}_{n}")
                                    for k in range(CT):
                                        nc.tensor.matmul(
                                            vps[:], hT[k][:, ts(s, P)],
                                            wv_sb[k][:, ts(n, CH)],
                                            start=(k == 0), stop=(k == CT - 1))
                                    nc.scalar.copy(Vall[s][:, ts(n, CH)], vps[:])

                        # ============ Phase 3: per-head attention ==========
                        with tc.tile_pool(name="p3", bufs=1) as p3:
                            for h in range(H):
                                o_nrm.append(opool.tile(
                                    [P, T], bf16, tag=f"o{h}", name=f"on_{h}"))
                                wqh = p3.tile([P, CT, P], bf16, tag="wqh",
                                              bufs=2, name=f"wqh_{h}")
                                nc.sync.dma_start(wqh[:], wq_d[h])
                                wkh = p3.tile([P, CT, P], bf16, tag="wkh",
                                              bufs=2, name=f"wkh_{h}")
                                nc.sync.dma_start(wkh[:], wk_d[h])
                                qT = p3.tile([P, T], bf16, tag="qT", bufs=2,
                                             name=f"qT_{h}")
                                kT = p3.tile([P, T], bf16, tag="kT", bufs=2,
                                             name=f"kT_{h}")
                                for j in range(NCH):
                                    sl = slice(j * CH, (j + 1) * CH)
                                    qps = ppw.tile([P, CH], fp32, tag="w",
                                                   name=f"q_ps_{h}_{j}")
                                    for k in range(CT):
                                        nc.tensor.matmul(qps[:], wqh[:, k, :],
                                                         hT[k][:, sl],
                                                         start=(k == 0),
                                                         stop=(k == CT - 1))
                                    nc.scalar.activation(qT[:, sl], qps[:],
                                                         AF.Identity,
                                                         bias=bq_sb[:, h:h + 1],
                                                         scale=1.0)
                                    kps = ppw.tile([P, CH], fp32, tag="w",
                                                   name=f"k_ps_{h}_{j}")
                                    for k in range(CT):
                                        nc.tensor.matmul(kps[:], wkh[:, k, :],
                                                         hT[k][:, sl],
                                                         start=(k == 0),
                                                         stop=(k == CT - 1))
                                    nc.scalar.copy(kT[:, sl], kps[:])
                                # scores + exp (causal: s-tile covers t >= s*P)
                                aT = []
                                for s in range(ST):
                                    at = p3.tile([P, T], bf16, tag=f"a{s}",
                                                 bufs=1, name=f"aT_{h}_{s}")
                                    aT.append(at)
                                    for j in range(NCH):
                                        lo = max(j * CH, s * P)
                                        hi = (j + 1) * CH
                                        if lo >= hi:
                                            continue
                                        sps = ppw.tile([P, CH], fp32, tag="w",
                                                       name=f"s_ps_{h}_{s}_{j}")
                                        nc.tensor.matmul(sps[:, :hi - lo],
                                                         kT[:, ts(s, P)],
                                                         qT[:, lo:hi],
                                                         start=True, stop=True)
                                        nc.scalar.activation(at[:, lo:hi],
                                                             sps[:, :hi - lo],
                                                             AF.Exp)
                                    nc.vector.tensor_mul(at[:, ts(s, P)],
                                                         at[:, ts(s, P)],
                                                         trimask[:])
                                # denominators: packed psum, row 0 (j=0)
                                # and row 32 (j=1)
                                den_ps = pps.tile([P, CH], fp32, tag="st",
                                                  bufs=2, name=f"dn_{h}")
                                for j in range(NCH):
                                    r0 = 32 * j
                                    smax = min(ST, 4 * (j + 1))
                                    for s in range(smax):
                                        lo = max(0, s * P - j * CH)
                                        nc.tensor.matmul(
                                            den_ps[r0:r0 + 1, lo:CH],
                                            ones_bf[:],
                                            aT[s][:, j * CH + lo:(j + 1) * CH],
                                            start=(s == 0), stop=(s == smax - 1))
                                # r_bc = 1/den broadcast: copy row, K=1 mm,
                                # then wide reciprocal straight off PSUM
                                r_bc = p3.tile([P, T], fp32, tag="rbc", bufs=2,
                                               name=f"rbc_{h}")
                                for j in range(NCH):
                                    dj = p3.tile([1, CH], fp32, tag="den",
                                                 bufs=2, name=f"den_{h}_{j}")
                                    nc.scalar.copy(
                                        dj[:], den_ps[32 * j:32 * j + 1, :])
                                    dps = bc_mm(dj[:], f"dbc_{h}_{j}")
                                    nc.vector.reciprocal_approx_fast(
                                        r_bc[:, j * CH:(j + 1) * CH], dps[:])
                                # attention @ V, then normalize
                                for j in range(NCH):
                                    smax = min(ST, 4 * (j + 1))
                                    ops_ = ppw.tile([P, CH], fp32, tag="w",
                                                    name=f"o_ps_{h}_{j}")
                                    for s in range(smax):
                                        lo = max(0, s * P - j * CH)
                                        nc.tensor.matmul(
                                            ops_[:, lo:CH],
                                            Vall[s][:, ts(h, P)],
                                            aT[s][:, j * CH + lo:(j + 1) * CH],
                                            start=(s == 0), stop=(s == smax - 1))
                                    sl = slice(j * CH, (j + 1) * CH)
                                    nc.vector.tensor_mul(o_nrm[h][:, sl],
                                                         ops_[:], r_bc[:, sl])

                # hT closed; Phase 4: out-proj + residual + BN1 stats
                # (j-outer so chunk 0's AllReduce overlaps chunk 1's matmuls)
                stp_bn1 = stat_tiles("bn1")
                bn1_io = [None, None]
                with tc.tile_pool(name="p4", bufs=1) as p4:
                    wok_sb = []
                    x2_sb = []
                    for k in range(CT):
                        wok = p4.tile([P, H, P], bf16, tag=f"wok{k}",
                                      name=f"wok_{k}")
                        nc.sync.dma_start(wok[:], wo_d[k])
                        wok_sb.append(wok)
                        x2k = p4.tile([P, T], fp32, tag=f"x2{k}",
                                      name=f"x2_{k}")
                        nc.sync.dma_start(x2k[:], xT_d[ts(k, P), :])
                        x2_sb.append(x2k)
                        u1.append(u1pool.tile([P, T], fp32, tag=f"u{k}",
                                              name=f"u1_{k}"))
                    for j in range(NCH):
                        sl = slice(j * CH, (j + 1) * CH)
                        for k in range(CT):
                            saps = ppw.tile([P, CH], fp32, tag="w",
                                            name=f"sa_ps_{k}_{j}")
                            for hh in range(H):
                                nc.tensor.matmul(saps[:], wok_sb[k][:, hh, :],
                                                 o_nrm[hh][:, sl],
                                                 start=(hh == 0),
                                                 stop=(hh == H - 1))
                            nc.vector.scalar_tensor_tensor(
                                out=u1[k][:, sl], in0=saps[:],
                                scalar=bo_sb[:, k:k + 1], in1=x2_sb[k][:, sl],
                                op0=OP.add, op1=OP.add)
                            stats_chunk(u1[k][:, sl], stp_bn1[j],
                                        k == 0, k == CT - 1)
                        bn1_io[j] = allreduce_chunk(u1pool, stp_bn1[j],
                                                    f"bn1_{j}")

            # ================= Phase 5: BN1 + LN2 =================
            with tc.tile_pool(name="h2T", bufs=1) as h2pool:
                h2T = [h2pool.tile([P, T], bf16, tag=f"h2{k}", name=f"h2_{k}")
                       for k in range(CT)]
                stp_bn2 = stat_tiles("bn2")
                bn2_io = [None, None]
                with tc.tile_pool(name="p5", bufs=1) as p5:
                    bn1g_sb = p5.tile([1, T], fp32, name="bn1g_sb")
                    nc.sync.dma_start(bn1g_sb[:], bn1g_d[:])
                    bn1b_sb = p5.tile([1, T], fp32, name="bn1b_sb")
                    nc.sync.dma_start(bn1b_sb[:], bn1b_d[:])
                    g1bc = broadcast_row_full(p5, bn1g_sb, "bn1g_bc", tag="gbc")
                    b1bc = broadcast_row_full(p5, bn1b_sb, "bn1b_bc", tag="bbc")
                    bn1_sc = []
                    bn1_bi = []
                    for j in range(NCH):
                        sl = slice(j * CH, (j + 1) * CH)
                        loc_j, glob_j = bn1_io[j]
                        sc, bi = norm_params_chunk(
                            p5, glob_j[:, 0:CH], glob_j[:, CH:2 * CH],
                            NBC, f"bn1_{j}", g_bc_sl=g1bc[:, sl],
                            b_bc_sl=b1bc[:, sl], sc_tag="bnsc", bi_tag="bnbi")
                        bn1_sc.append(sc)
                        bn1_bi.append(bi)
                        # LN2(BN1(u1)) == u1*A + B with A = s*rstd2,
                        # B = -mean_c(u1)*A, rstd2 = 1/sqrt(s^2*var_c(u1)+eps)
                        # -- derived from the LOCAL per-core stats rows, so no
                        # second stats pass is needed.
                        mc = p5.tile([1, CH], fp32, tag="rm", bufs=2,
                                     name=f"ln2m_{j}")
                        nc.vector.tensor_scalar_mul(mc[:], loc_j[:, 0:CH],
                                                    1.0 / C)
                        vc = p5.tile([1, CH], fp32, tag="rv", bufs=2,
                                     name=f"ln2v_{j}")
                        nc.vector.tensor_scalar_mul(vc[:], loc_j[:, CH:2 * CH],
                                                    1.0 / C)
                        msq = p5.tile([1, CH], fp32, tag="rq", bufs=2,
                                      name=f"ln2q_{j}")
                        nc.vector.tensor_mul(msq[:], mc[:], mc[:])
                        nc.vector.tensor_sub(vc[:], vc[:], msq[:])
                        mc_ps = bc_mm(mc[:], f"ln2mb_{j}")
                        vc_ps = bc_mm(vc[:], f"ln2vb_{j}")
                        s2t = spool.tile([P, CH], fp32, tag="nst", bufs=1,
                                         name=f"ln2s2_{j}")
                        nc.vector.tensor_mul(s2t[:], sc[:], sc[:])
                        v2 = spool.tile([P, CH], fp32, tag="nmt", bufs=1,
                                        name=f"ln2v2_{j}")
                        nc.vector.tensor_mul(v2[:], s2t[:], vc_ps[:])
                        st2 = spool.tile([P, CH], fp32, tag="nst", bufs=1,
                                         name=f"ln2st_{j}")
                        nc.scalar.activation(st2[:], v2[:], AF.Sqrt,
                                             bias=eps_col[:, 0:1], scale=1.0)
                        r2 = spool.tile([P, CH], fp32, tag="nrc", bufs=1,
                                        name=f"ln2r_{j}")
                        nc.vector.reciprocal_approx_fast(r2[:], st2[:])
                        A = p5.tile([P, CH], fp32, tag="nsc", bufs=2,
                                    name=f"ln2A_{j}")
                        nc.vector.tensor_mul(A[:], r2[:], sc[:])
                        mt2 = spool.tile([P, CH], fp32, tag="nmt", bufs=1,
                                         name=f"ln2mt_{j}")
                        nc.vector.tensor_mul(mt2[:], mc_ps[:], A[:])
                        Bt = p5.tile([P, CH], fp32, tag="nbi", bufs=2,
                                     name=f"ln2B_{j}")
                        nc.vector.tensor_scalar_mul(Bt[:], mt2[:], -1.0)
                        for k in range(CT):
                            affine_chunk(h2T[k][:, sl], u1[k][:, sl],
                                         A[:], Bt[:])
                    # deferred u1 -> BN1(u1) affines; these only gate the
                    # phase-6 residual adds, so they execute on DVE slack
                    # while the FFN matmuls run.
                    for j in range(NCH):
                        sl = slice(j * CH, (j + 1) * CH)
                        for k in range(CT):
                            affine_chunk(u1[k][:, sl], u1[k][:, sl],
                                         bn1_sc[j][:], bn1_bi[j][:])

                # ================= Phase 6: FFN =================
                with tc.tile_pool(name="p6", bufs=1) as p6:
                    for j in range(NCH):
                        sl = slice(j * CH, (j + 1) * CH)
                        z = []
                        for f in range(FT):
                            w1f = p6.tile([P, CT, P], bf16, tag="w1f", bufs=2,
                                          name=f"w1f_{j}_{f}")
                            nc.sync.dma_start(w1f[:], w1_d[f])
                            zps = ppw.tile([P, CH], fp32, tag="w",
                                           name=f"z_ps_{j}_{f}")
                            for k in range(CT):
                                nc.tensor.matmul(zps[:], w1f[:, k, :],
                                                 h2T[k][:, sl],
                                                 start=(k == 0),
                                                 stop=(k == CT - 1))
                            zf = p6.tile([P, CH], bf16, tag=f"z{f}",
                                         name=f"z_{j}_{f}")
                            nc.scalar.activation(zf[:], zps[:], AF.Relu,
                                                 bias=b1_sb[:, f:f + 1],
                                                 scale=1.0)
                            z.append(zf)
                        for k in range(CT):
                            w2k = p6.tile([P, FT, P], bf16, tag="w2k", bufs=2,
                                          name=f"w2k_{j}_{k}")
                            nc.sync.dma_start(w2k[:], w2_d[k])
                            yps = ppw.tile([P, CH], fp32, tag="w",
                                           name=f"y_ps_{j}_{k}")
                            for f in range(FT):
                                nc.tensor.matmul(yps[:], w2k[:, f, :], z[f][:],
                                                 start=(f == 0),
                                                 stop=(f == FT - 1))
                            nc.vector.scalar_tensor_tensor(
                                out=u1[k][:, sl], in0=yps[:],
                                scalar=b2_sb[:, k:k + 1], in1=u1[k][:, sl],
                                op0=OP.add, op1=OP.add)
                            stats_chunk(u1[k][:, sl], stp_bn2[j],
                                        k == 0, k == CT - 1)
                        bn2_io[j] = allreduce_chunk(u1pool, stp_bn2[j],
                                                    f"bn2_{j}")

            # ================= Phase 7: BN2 + output =================
            with tc.tile_pool(name="p7", bufs=1) as p7:
                bn2g_sb = p7.tile([1, T], fp32, name="bn2g_sb")
                nc.sync.dma_start(bn2g_sb[:], bn2g_d[:])
                bn2b_sb = p7.tile([1, T], fp32, name="bn2b_sb")
                nc.sync.dma_start(bn2b_sb[:], bn2b_d[:])
                g2bc = broadcast_row_full(p7, bn2g_sb, "bn2g_bc", tag="gbc")
                b2bc = broadcast_row_full(p7, bn2b_sb, "bn2b_bc", tag="bbc")
                for j in range(NCH):
                    sl = slice(j * CH, (j + 1) * CH)
                    sc, bi = norm_params_chunk(
                        p7, bn2_io[j][1][:, 0:CH], bn2_io[j][1][:, CH:2 * CH],
                        NBC, f"bn2_{j}", g_bc_sl=g2bc[:, sl],
                        b_bc_sl=b2bc[:, sl])
                    for k in range(CT):
                        tmp = spool.tile([P, CH], fp32, tag="ntmp", bufs=2,
                                         name="ytmp")
                        nc.vector.tensor_mul(tmp[:], u1[k][:, sl], sc[:])
                        yk = spool.tile([P, CH], fp32, tag="yout", bufs=2,
                                        name=f"y_{k}_{j}")
                        nc.vector.tensor_add(yk[:], tmp[:], bi[:])
                        nc.sync.dma_start(yT_d[ts(k, P), sl], yk[:])

    nc.compile()
    return nc


def _get_program():
    global _PROG
    if _PROG is None:
        _PROG = _build()
    return _PROG


def _prep_shared(inputs):
    """Host-side weight folding + pre-tiling; identical for every core."""
    f32 = np.float32
    bf16 = ml_dtypes.bfloat16
    wq = np.asarray(inputs["wq"], f32)      # (H, C, D)
    wk = np.asarray(inputs["wk"], f32)
    wv = np.asarray(inputs["wv"], f32)
    wo = np.asarray(inputs["wo"], f32)      # (C, C)
    bo = np.asarray(inputs["bo"], f32)      # (C,)
    g1 = np.asarray(inputs["ln1_g"], f32)
    b1n = np.asarray(inputs["ln1_b"], f32)
    g2 = np.asarray(inputs["ln2_g"], f32)
    b2n = np.asarray(inputs["ln2_b"], f32)
    w1 = np.asarray(inputs["w1"], f32)      # (C, F)
    b1 = np.asarray(inputs["b1"], f32)      # (F,)
    w2 = np.asarray(inputs["w2"], f32)      # (F, C)
    b2 = np.asarray(inputs["b2"], f32)      # (C,)

    dscale = f32(D) ** f32(-0.5)
    # fold ln1 affine into qkv projections; q also takes 1/sqrt(D)
    wq2 = (wq * g1[None, :, None] * dscale).transpose(1, 0, 2).reshape(C, C)
    wk2 = (wk * g1[None, :, None]).transpose(1, 0, 2).reshape(C, C)
    wv2 = (wv * g1[None, :, None]).transpose(1, 0, 2).reshape(C, C)
    bq = (np.einsum("c,hcd->hd", b1n, wq) * dscale).reshape(C)
    bv = np.einsum("c,hcd->hd", b1n, wv).reshape(C)
    # k-side bias cancels in softmax (constant per row); v bias folds into bo
    bo2 = bo + bv @ wo
    w1f = g2[:, None] * w1
    b1f = b1 + b2n @ w1

    def lhst_tiles(w, n_out):
        # (C_in, n_out*P) -> (n_out, P, C_in//P, P):
        # [o, p, ki, n] = w[ki*P + p, o*P + n]
        ci = w.shape[0]
        return np.ascontiguousarray(
            w.reshape(ci // P, P, n_out, P).transpose(2, 1, 0, 3)
        ).astype(bf16)

    def cols(v, n):  # (n*P,) -> (P, n) with [p, i] = v[i*P + p]
        return np.ascontiguousarray(v.reshape(n, P).T, dtype=f32)

    def row(v):
        return np.ascontiguousarray(v.reshape(1, T), dtype=f32)

    return dict(
        wq=lhst_tiles(wq2, H), wk=lhst_tiles(wk2, H),
        wv=wv2.astype(bf16),
        bq=cols(bq, H), wo=lhst_tiles(wo, CT), bo=cols(bo2, CT),
        w1=lhst_tiles(w1f, FT), b1=cols(b1f, FT),
        w2=lhst_tiles(w2, CT), b2=cols(b2, CT),
        bn1g=row(np.asarray(inputs["bn1_g"], f32)),
        bn1b=row(np.asarray(inputs["bn1_b"], f32)),
        bn2g=row(np.asarray(inputs["bn2_g"], f32)),
        bn2b=row(np.asarray(inputs["bn2_b"], f32)),
    )


def _run(inputs, trace=False):
    from concourse import bass_utils
    nc = _get_program()
    x = np.asarray(inputs["x"], np.float32)
    shared = _prep_shared(inputs)
    in_maps = []
    for b in range(B):
        m = dict(shared)
        xt = np.ascontiguousarray(x[b].T)
        m["xT"] = xt
        m["xbf"] = xt.astype(ml_dtypes.bfloat16)
        in_maps.append(m)
    res = bass_utils.run_bass_kernel_spmd(
        nc, in_maps, core_ids=list(range(NCORES)), trace=trace)
    out = np.stack([res.results[b]["yT"].T for b in range(B)]).astype(np.float32)
    return out, res


def kernel(**inputs):
    out, _ = _run(inputs, trace=False)
    return out


# revision 15
# speedup vs baseline: 1.1791x; 1.0126x over previous
"""Trainium2 Bass kernel for nn_Block_50113678410401 (dense transformer block).

Strategy: data-parallel over the batch axis (B=8 -> 8 NeuronCores, one batch
element per core). All on-chip activations live in "layout A": feature axis on
SBUF partitions, token axis (T) on the free dimension, so no on-chip
transposes are needed (host pre-transposes x and post-transposes the output).

Per core:
  LN1 (stats via ones-matmul over partitions), per-head causal attention
  (no-max-sub exp softmax, denominator via ones-matmul, normalization via
  K=1 broadcast matmul + wide reciprocal), output projection + residual,
  BatchNorm over (B,C) with per-512-chunk cross-core AllReduces of
  (sum, sumsq) per T channel (the first chunk's collective overlaps the
  second chunk's matmuls), LN2, FFN (C -> 4C -> relu -> C), residual,
  second BatchNorm.

All big matmuls run in bf16 with fp32 PSUM accumulation; statistics,
softmax, residuals and normalizations are fp32. Weights arrive host-pretiled
so every weight DMA is contiguous per partition.

LayerNorm/projection affine parameters are folded into the weights on the
host: wq' = diag(ln1_g) wq / sqrt(D) (q also carries 1/sqrt(D)), k-side bias
drops out of softmax by shift invariance, v-side bias is folded into the
output-projection bias, ln2 affine is folded into w1/b1.
"""

import numpy as np
import ml_dtypes

B, T, C, H, D = 8, 1024, 1536, 12, 128
F = 4 * C            # 6144
P = 128
CT = C // P          # 12 c-tiles
FT = F // P          # 48 f-tiles
ST = T // P          # 8 s-tiles
CH = 512             # matmul free-dim chunk
NCH = T // CH        # 2 chunks
EPS = 1e-5
NCORES = 8
NBC = B * C          # BatchNorm count over (B, C)

_PROG = None


def _build():
    import concourse.bass as bass
    import concourse.mybir as mybir
    import concourse.tile as tile
    from concourse import bacc
    from concourse.masks import make_upper_triangular

    fp32 = mybir.dt.float32
    bf16 = mybir.dt.bfloat16
    AF = mybir.ActivationFunctionType
    OP = mybir.AluOpType
    ts = bass.ts

    nc = bacc.Bacc("TRN2", target_bir_lowering=False, debug=False,
                   enable_asserts=True, num_devices=NCORES)

    # ---- DRAM I/O (weights host-pretiled for contiguous DMA) ----
    xT_d = nc.dram_tensor("xT", (C, T), fp32, kind="ExternalInput").ap()
    xbf_d = nc.dram_tensor("xbf", (C, T), bf16, kind="ExternalInput").ap()
    wq_d = nc.dram_tensor("wq", (H, P, CT, P), bf16, kind="ExternalInput").ap()
    wk_d = nc.dram_tensor("wk", (H, P, CT, P), bf16, kind="ExternalInput").ap()
    wv_d = nc.dram_tensor("wv", (C, C), bf16, kind="ExternalInput").ap()
    bq_d = nc.dram_tensor("bq", (P, H), fp32, kind="ExternalInput").ap()
    wo_d = nc.dram_tensor("wo", (CT, P, H, P), bf16, kind="ExternalInput").ap()
    bo_d = nc.dram_tensor("bo", (P, CT), fp32, kind="ExternalInput").ap()
    w1_d = nc.dram_tensor("w1", (FT, P, CT, P), bf16, kind="ExternalInput").ap()
    b1_d = nc.dram_tensor("b1", (P, FT), fp32, kind="ExternalInput").ap()
    w2_d = nc.dram_tensor("w2", (CT, P, FT, P), bf16, kind="ExternalInput").ap()
    b2_d = nc.dram_tensor("b2", (P, CT), fp32, kind="ExternalInput").ap()
    bn1g_d = nc.dram_tensor("bn1g", (1, T), fp32, kind="ExternalInput").ap()
    bn1b_d = nc.dram_tensor("bn1b", (1, T), fp32, kind="ExternalInput").ap()
    bn2g_d = nc.dram_tensor("bn2g", (1, T), fp32, kind="ExternalInput").ap()
    bn2b_d = nc.dram_tensor("bn2b", (1, T), fp32, kind="ExternalInput").ap()
    yT_d = nc.dram_tensor("yT", (C, T), fp32, kind="ExternalOutput").ap()

    with tile.TileContext(nc) as tc:
        with tc.tile_pool(name="const", bufs=1) as cpool, \
             tc.tile_pool(name="scratch", bufs=1) as spool, \
             tc.tile_pool(name="u1p", bufs=1) as u1pool, \
             tc.tile_pool(name="ppw", bufs=6, space="PSUM") as ppw, \
             tc.tile_pool(name="pps", bufs=2, space="PSUM") as pps, \
             tc.tile_pool(name="dram", bufs=1, space="DRAM") as dpool:

            # ---- constants ----
            ones_bf = cpool.tile([P, 1], bf16, name="ones_bf")
            nc.vector.memset(ones_bf[:], 1.0)
            trimask = cpool.tile([P, P], bf16, name="trimask")
            make_upper_triangular(nc, trimask[:], val=1.0, diag=True)
            bq_sb = cpool.tile([P, H], fp32, name="bq_sb")
            nc.sync.dma_start(bq_sb[:], bq_d[:])
            bo_sb = cpool.tile([P, CT], fp32, name="bo_sb")
            nc.sync.dma_start(bo_sb[:], bo_d[:])
            b1_sb = cpool.tile([P, FT], fp32, name="b1_sb")
            nc.sync.dma_start(b1_sb[:], b1_d[:])
            b2_sb = cpool.tile([P, CT], fp32, name="b2_sb")
            nc.sync.dma_start(b2_sb[:], b2_d[:])

            # ---- helpers ----
            def bcast_into(dst_ap, row_ap, name, n=CH):
                """(1, n) fp32 SBUF row -> (P, n) via DRAM bounce; DMAs ride
                the gpsimd queue so they never block the weight loads."""
                dr = dpool.tile([1, n], fp32, tag="bcd", bufs=4,
                                name=f"{name}_dr")
                nc.gpsimd.dma_start(dr[:], row_ap)
                nc.gpsimd.dma_start(dst_ap, dr[:].to_broadcast((P, n)))

            # Packed stat psum tile: row 0 accumulates sum, row 32 sumsq.
            def stat_tiles(name):
                return [pps.tile([P, CH], fp32, tag="st", bufs=2,
                                 name=f"{name}_{j}") for j in range(NCH)]

            def stats_chunk(src_ap, stp_j, first, last, is_bf16=False):
                """Ones-matmul partial sums of src chunk ((P,CH)) and its
                square into packed stat rows."""
                if is_bf16:
                    cbf = src_ap
                else:
                    cbf_t = spool.tile([P, CH], bf16, tag="cast_bf", bufs=2,
                                       name="cbf")
                    nc.vector.tensor_copy(cbf_t[:], src_ap)
                    cbf = cbf_t[:]
                csq = spool.tile([P, CH], bf16, tag="cast_sq", bufs=2,
                                 name="csq")
                nc.scalar.square(csq[:], src_ap)
                nc.tensor.matmul(stp_j[0:1, :], ones_bf[:], cbf,
                                 start=first, stop=last)
                nc.tensor.matmul(stp_j[32:33, :], ones_bf[:], csq[:],
                                 start=first, stop=last)

            def allreduce_chunk(pool, stp_j, name):
                """AllReduce-add this chunk's packed (sum, sumsq) across
                cores. Returns the (1, 2*CH) global row."""
                loc = pool.tile([1, 2 * CH], fp32, tag="arloc", bufs=2,
                                name=f"{name}_loc")
                nc.scalar.copy(loc[:, 0:CH], stp_j[0:1, :])
                nc.scalar.copy(loc[:, CH:2 * CH], stp_j[32:33, :])
                cin = dpool.tile([1, 2 * CH], fp32, name=f"{name}_cin")
                cout = dpool.tile([1, 2 * CH], fp32, name=f"{name}_cout")
                nc.gpsimd.dma_start(cin[:], loc[:])
                nc.gpsimd.collective_compute(
                    "AllReduce", mybir.AluOpType.add,
                    replica_groups=[list(range(NCORES))],
                    ins=[cin.opt()], outs=[cout.opt()],
                )
                glob = pool.tile([1, 2 * CH], fp32, tag="arglob", bufs=2,
                                 name=f"{name}_glob")
                nc.gpsimd.dma_start(glob[:], cout[:])
                return loc, glob

            def norm_params_chunk(pool, s1_ap, s2_ap, count, name,
                                  g_row_sl=None, b_row_sl=None,
                                  sc_tag="nsc", bi_tag="nbi"):
                """Per-chunk normalization params, computed at row level
                (single-lane, cheap custom-DVE reciprocal), then broadcast
                to (P, CH) via DRAM-bounce DMA. Returns (sc_bc, bi_bc,
                scale_row, mean_row)."""
                m = pool.tile([1, CH], fp32, tag="rm", bufs=2,
                              name=f"{name}_m")
                nc.vector.tensor_scalar_mul(m[:], s1_ap, 1.0 / count)
                v = pool.tile([1, CH], fp32, tag="rv", bufs=2,
                              name=f"{name}_v")
                nc.vector.tensor_scalar_mul(v[:], s2_ap, 1.0 / count)
                msq = pool.tile([1, CH], fp32, tag="rq", bufs=2,
                                name=f"{name}_msq")
                nc.vector.tensor_mul(msq[:], m[:], m[:])
                nc.vector.tensor_sub(v[:], v[:], msq[:])
                nc.vector.tensor_scalar_add(v[:], v[:], EPS)
                nc.scalar.sqrt(v[:], v[:])
                scale = pool.tile([1, CH], fp32, tag="rs", bufs=2,
                                  name=f"{name}_srow")
                if g_row_sl is not None:
                    rc = pool.tile([1, CH], fp32, tag="rr", bufs=2,
                                   name=f"{name}_rc")
                    nc.vector.reciprocal_approx_fast(rc[:], v[:])
                    nc.vector.tensor_mul(scale[:], rc[:], g_row_sl)
                else:
                    nc.vector.reciprocal_approx_fast(scale[:], v[:])
                bias = pool.tile([1, CH], fp32, tag="rb", bufs=2,
                                 name=f"{name}_brow")
                nc.vector.tensor_mul(bias[:], m[:], scale[:])
                nc.vector.tensor_scalar_mul(bias[:], bias[:], -1.0)
                if b_row_sl is not None:
                    nc.vector.tensor_add(bias[:], bias[:], b_row_sl)
                sc_bc = pool.tile([P, CH], fp32, tag=sc_tag, bufs=2,
                                  name=f"{name}_scbc")
                bcast_into(sc_bc[:], scale[:], f"{name}_sc")
                bi_bc = pool.tile([P, CH], fp32, tag=bi_tag, bufs=2,
                                  name=f"{name}_bibc")
                bcast_into(bi_bc[:], bias[:], f"{name}_bi")
                return sc_bc, bi_bc, scale, m

            def affine_chunk(dst_ap, src_ap, sc_ap, bi_ap):
                """dst = src * sc + bi on one (P, CH) chunk."""
                tmp = spool.tile([P, CH], fp32, tag="ntmp", bufs=2,
                                 name="ntmp")
                nc.vector.tensor_mul(tmp[:], src_ap, sc_ap)
                nc.vector.tensor_add(dst_ap, tmp[:], bi_ap)

            u1 = []     # created at phase 4 (first use)
            o_nrm = []  # created at phase 3

            with tc.tile_pool(name="onrm", bufs=1) as opool:
                with tc.tile_pool(name="hT", bufs=1) as hpool:
                    hT = [hpool.tile([P, T], bf16, tag=f"h{k}", name=f"hT_{k}")
                          for k in range(CT)]
                    # ================= Phase 1: LN1 =================
                    with tc.tile_pool(name="p1", bufs=1) as p1:
                        stp = stat_tiles("ln1")
                        x_sb = []
                        for k in range(CT):
                            xk = p1.tile([P, T], bf16, tag=f"x{k}", name=f"x_{k}")
                            nc.sync.dma_start(xk[:], xbf_d[ts(k, P), :])
                            x_sb.append(xk)
                            for j in range(NCH):
                                sl = slice(j * CH, (j + 1) * CH)
                                stats_chunk(xk[:, sl], stp[j], k == 0,
                                            k == CT - 1, is_bf16=True)
                        for j in range(NCH):
                            sl = slice(j * CH, (j + 1) * CH)
                            sc, bi, _, _ = norm_params_chunk(
                                p1, stp[j][0:1, :], stp[j][32:33, :], C,
                                f"ln1_{j}")
                            for k in range(CT):
                                affine_chunk(hT[k][:, sl], x_sb[k][:, sl],
                                             sc[:], bi[:])

                    # ================= Phase 2: V for all heads ============
                    with tc.tile_pool(name="vall", bufs=1) as vpool:
                        Vall = [vpool.tile([P, C], bf16, tag=f"v{s}",
                                           name=f"V_{s}") for s in range(ST)]
                        with tc.tile_pool(name="wv", bufs=1) as wvpool:
                            wv_sb = []
                            for k in range(CT):
                                wvk = wvpool.tile([P, C], bf16, tag=f"wv{k}",
                                                  name=f"wv_{k}")
                                nc.sync.dma_start(wvk[:], wv_d[ts(k, P), :])
                                wv_sb.append(wvk)
                            for s in range(ST):
                                for n in range(C // CH):
                                    vps = ppw.tile([P, CH], fp32, tag="w",
                                                   name=f"v_ps_{s}_{n}")
                                    for k in range(CT):
                                        nc.tensor.matmul(
                                            vps[:], hT[k][:, ts(s, P)],
                                            wv_sb[k][:, ts(n, CH)],
                                            start=(k == 0), stop=(k == CT - 1))
                                    nc.scalar.copy(Vall[s][:, ts(n, CH)], vps[:])

                        # ============ Phase 3: per-head attention ==========
                        with tc.tile_pool(name="p3", bufs=1) as p3:
                            for h in range(H):
                                o_nrm.append(opool.tile(
                                    [P, T], bf16, tag=f"o{h}", name=f"on_{h}"))
                                wqh = p3.tile([P, CT, P], bf16, tag="wqh",
                                              bufs=2, name=f"wqh_{h}")
                                nc.sync.dma_start(wqh[:], wq_d[h])
                                wkh = p3.tile([P, CT, P], bf16, tag="wkh",
                                              bufs=2, name=f"wkh_{h}")
                                nc.sync.dma_start(wkh[:], wk_d[h])
                                qT = p3.tile([P, T], bf16, tag="qT", bufs=2,
                                             name=f"qT_{h}")
                                kT = p3.tile([P, T], bf16, tag="kT", bufs=2,
                                             name=f"kT_{h}")
                                for j in range(NCH):
                                    sl = slice(j * CH, (j + 1) * CH)
                                    qps = ppw.tile([P, CH], fp32, tag="w",
                                                   name=f"q_ps_{h}_{j}")
                                    for k in range(CT):
                                        nc.tensor.matmul(qps[:], wqh[:, k, :],
                                                         hT[k][:, sl],
                                                         start=(k == 0),
                                                         stop=(k == CT - 1))
                                    nc.scalar.activation(qT[:, sl], qps[:],
                                                         AF.Identity,
                                                         bias=bq_sb[:, h:h + 1],
                                                         scale=1.0)
                                    kps = ppw.tile([P, CH], fp32, tag="w",
                                                   name=f"k_ps_{h}_{j}")
                                    for k in range(CT):
                                        nc.tensor.matmul(kps[:], wkh[:, k, :],
                                                         hT[k][:, sl],
                                                         start=(k == 0),
                                                         stop=(k == CT - 1))
                                    nc.scalar.copy(kT[:, sl], kps[:])
                                # scores + exp (causal: s-tile covers t >= s*P)
                                aT = []
                                for s in range(ST):
                                    at = p3.tile([P, T], bf16, tag=f"a{s}",
                                                 bufs=1, name=f"aT_{h}_{s}")
                                    aT.append(at)
                                    for j in range(NCH):
                                        lo = max(j * CH, s * P)
                                        hi = (j + 1) * CH
                                        if lo >= hi:
                                            continue
                                        sps = ppw.tile([P, CH], fp32, tag="w",
                                                       name=f"s_ps_{h}_{s}_{j}")
                                        nc.tensor.matmul(sps[:, :hi - lo],
                                                         kT[:, ts(s, P)],
                                                         qT[:, lo:hi],
                                                         start=True, stop=True)
                                        nc.scalar.activation(at[:, lo:hi],
                                                             sps[:, :hi - lo],
                                                             AF.Exp)
                                    nc.vector.tensor_mul(at[:, ts(s, P)],
                                                         at[:, ts(s, P)],
                                                         trimask[:])
                                # denominators: packed psum, row 0 (j=0)
                                # and row 32 (j=1)
                                den_ps = pps.tile([P, CH], fp32, tag="st",
                                                  bufs=2, name=f"dn_{h}")
                                for j in range(NCH):
                                    r0 = 32 * j
                                    smax = min(ST, 4 * (j + 1))
                                    for s in range(smax):
                                        lo = max(0, s * P - j * CH)
                                        nc.tensor.matmul(
                                            den_ps[r0:r0 + 1, lo:CH],
                                            ones_bf[:],
                                            aT[s][:, j * CH + lo:(j + 1) * CH],
                                            start=(s == 0), stop=(s == smax - 1))
                                # r_bc = 1/den broadcast: copy row, K=1 mm,
                                # then wide reciprocal straight off PSUM
                                r_bc = p3.tile([P, T], fp32, tag="rbc", bufs=2,
                                               name=f"rbc_{h}")
                                for j in range(NCH):
                                    dj = p3.tile([1, CH], fp32, tag="den",
                                                 bufs=2, name=f"den_{h}_{j}")
                                    nc.scalar.copy(
                                        dj[:], den_ps[32 * j:32 * j + 1, :])
                                    rj = p3.tile([1, CH], fp32, tag="rrow",
                                                 bufs=2, name=f"rr_{h}_{j}")
                                    nc.vector.reciprocal_approx_fast(
                                        rj[:], dj[:])
                                    bcast_into(r_bc[:, j * CH:(j + 1) * CH],
                                               rj[:], f"rbc_{h}_{j}")
                                # attention @ V, then normalize
                                for j in range(NCH):
                                    smax = min(ST, 4 * (j + 1))
                                    ops_ = ppw.tile([P, CH], fp32, tag="w",
                                                    name=f"o_ps_{h}_{j}")
                                    for s in range(smax):
                                        lo = max(0, s * P - j * CH)
                                        nc.tensor.matmul(
                                            ops_[:, lo:CH],
                                            Vall[s][:, ts(h, P)],
                                            aT[s][:, j * CH + lo:(j + 1) * CH],
                                            start=(s == 0), stop=(s == smax - 1))
                                    sl = slice(j * CH, (j + 1) * CH)
                                    nc.vector.tensor_mul(o_nrm[h][:, sl],
                                                         ops_[:], r_bc[:, sl])

                # hT closed; Phase 4: out-proj + residual + BN1 stats
                # (j-outer so chunk 0's AllReduce overlaps chunk 1's matmuls)
                stp_bn1 = stat_tiles("bn1")
                bn1_io = [None, None]
                with tc.tile_pool(name="p4", bufs=1) as p4:
                    wok_sb = []
                    x2_sb = []
                    for k in range(CT):
                        wok = p4.tile([P, H, P], bf16, tag=f"wok{k}",
                                      name=f"wok_{k}")
                        nc.sync.dma_start(wok[:], wo_d[k])
                        wok_sb.append(wok)
                        x2k = p4.tile([P, T], fp32, tag=f"x2{k}",
                                      name=f"x2_{k}")
                        nc.sync.dma_start(x2k[:], xT_d[ts(k, P), :])
                        x2_sb.append(x2k)
                        u1.append(u1pool.tile([P, T], fp32, tag=f"u{k}",
                                              name=f"u1_{k}"))
                    for j in range(NCH):
                        sl = slice(j * CH, (j + 1) * CH)
                        for k in range(CT):
                            saps = ppw.tile([P, CH], fp32, tag="w",
                                            name=f"sa_ps_{k}_{j}")
                            for hh in range(H):
                                nc.tensor.matmul(saps[:], wok_sb[k][:, hh, :],
                                                 o_nrm[hh][:, sl],
                                                 start=(hh == 0),
                                                 stop=(hh == H - 1))
                            nc.vector.scalar_tensor_tensor(
                                out=u1[k][:, sl], in0=saps[:],
                                scalar=bo_sb[:, k:k + 1], in1=x2_sb[k][:, sl],
                                op0=OP.add, op1=OP.add)
                            stats_chunk(u1[k][:, sl], stp_bn1[j],
                                        k == 0, k == CT - 1)
                        bn1_io[j] = allreduce_chunk(u1pool, stp_bn1[j],
                                                    f"bn1_{j}")

            # ================= Phase 5: BN1 + LN2 =================
            with tc.tile_pool(name="h2T", bufs=1) as h2pool:
                h2T = [h2pool.tile([P, T], bf16, tag=f"h2{k}", name=f"h2_{k}")
                       for k in range(CT)]
                stp_bn2 = stat_tiles("bn2")
                bn2_io = [None, None]
                with tc.tile_pool(name="p5", bufs=1) as p5:
                    bn1g_sb = p5.tile([1, T], fp32, name="bn1g_sb")
                    nc.sync.dma_start(bn1g_sb[:], bn1g_d[:])
                    bn1b_sb = p5.tile([1, T], fp32, name="bn1b_sb")
                    nc.sync.dma_start(bn1b_sb[:], bn1b_d[:])
                    bn1_sc = []
                    bn1_bi = []
                    for j in range(NCH):
                        sl = slice(j * CH, (j + 1) * CH)
                        loc_j, glob_j = bn1_io[j]
                        sc, bi, s_row, _ = norm_params_chunk(
                            p5, glob_j[:, 0:CH], glob_j[:, CH:2 * CH],
                            NBC, f"bn1_{j}", g_row_sl=bn1g_sb[:, sl],
                            b_row_sl=bn1b_sb[:, sl], sc_tag="bnsc",
                            bi_tag="bnbi")
                        bn1_sc.append(sc)
                        bn1_bi.append(bi)
                        # LN2(BN1(u1)) == u1*A + B with A = s*rstd2,
                        # B = -mean_c(u1)*A, rstd2 = 1/sqrt(s^2*var_c(u1)+eps)
                        # -- derived from the LOCAL per-core stats rows, so no
                        # second stats pass is needed.
                        mc = p5.tile([1, CH], fp32, tag="rm", bufs=2,
                                     name=f"ln2m_{j}")
                        nc.vector.tensor_scalar_mul(mc[:], loc_j[:, 0:CH],
                                                    1.0 / C)
                        vc = p5.tile([1, CH], fp32, tag="rv", bufs=2,
                                     name=f"ln2v_{j}")
                        nc.vector.tensor_scalar_mul(vc[:], loc_j[:, CH:2 * CH],
                                                    1.0 / C)
                        msq = p5.tile([1, CH], fp32, tag="rq", bufs=2,
                                      name=f"ln2q_{j}")
                        nc.vector.tensor_mul(msq[:], mc[:], mc[:])
                        nc.vector.tensor_sub(vc[:], vc[:], msq[:])
                        s2r = p5.tile([1, CH], fp32, tag="rr", bufs=2,
                                      name=f"ln2s2_{j}")
                        nc.vector.tensor_mul(s2r[:], s_row[:], s_row[:])
                        nc.vector.tensor_mul(vc[:], vc[:], s2r[:])
                        nc.vector.tensor_scalar_add(vc[:], vc[:], EPS)
                        nc.scalar.sqrt(vc[:], vc[:])
                        r2 = p5.tile([1, CH], fp32, tag="rs", bufs=2,
                                     name=f"ln2r_{j}")
                        nc.vector.reciprocal_approx_fast(r2[:], vc[:])
                        arow = p5.tile([1, CH], fp32, tag="rb", bufs=2,
                                       name=f"ln2a_{j}")
                        nc.vector.tensor_mul(arow[:], r2[:], s_row[:])
                        brow = p5.tile([1, CH], fp32, tag="rb2", bufs=2,
                                       name=f"ln2b_{j}")
                        nc.vector.tensor_mul(brow[:], mc[:], arow[:])
                        nc.vector.tensor_scalar_mul(brow[:], brow[:], -1.0)
                        A = p5.tile([P, CH], fp32, tag="nsc", bufs=2,
                                    name=f"ln2A_{j}")
                        bcast_into(A[:], arow[:], f"ln2A_{j}")
                        Bt = p5.tile([P, CH], fp32, tag="nbi", bufs=2,
                                     name=f"ln2B_{j}")
                        bcast_into(Bt[:], brow[:], f"ln2B_{j}")
                        for k in range(CT):
                            affine_chunk(h2T[k][:, sl], u1[k][:, sl],
                                         A[:], Bt[:])
                    # deferred u1 -> BN1(u1) affines; these only gate the
                    # phase-6 residual adds, so they execute on DVE slack
                    # while the FFN matmuls run.
                    for j in range(NCH):
                        sl = slice(j * CH, (j + 1) * CH)
                        for k in range(CT):
                            affine_chunk(u1[k][:, sl], u1[k][:, sl],
                                         bn1_sc[j][:], bn1_bi[j][:])

                # ================= Phase 6: FFN =================
                with tc.tile_pool(name="p6", bufs=1) as p6:
                    for j in range(NCH):
                        sl = slice(j * CH, (j + 1) * CH)
                        z = []
                        for f in range(FT):
                            w1f = p6.tile([P, CT, P], bf16, tag="w1f", bufs=2,
                                          name=f"w1f_{j}_{f}")
                            nc.sync.dma_start(w1f[:], w1_d[f])
                            zps = ppw.tile([P, CH], fp32, tag="w",
                                           name=f"z_ps_{j}_{f}")
                            for k in range(CT):
                                nc.tensor.matmul(zps[:], w1f[:, k, :],
                                                 h2T[k][:, sl],
                                                 start=(k == 0),
                                                 stop=(k == CT - 1))
                            zf = p6.tile([P, CH], bf16, tag=f"z{f}",
                                         name=f"z_{j}_{f}")
                            nc.scalar.activation(zf[:], zps[:], AF.Relu,
                                                 bias=b1_sb[:, f:f + 1],
                                                 scale=1.0)
                            z.append(zf)
                        for k in range(CT):
                            w2k = p6.tile([P, FT, P], bf16, tag="w2k", bufs=2,
                                          name=f"w2k_{j}_{k}")
                            nc.sync.dma_start(w2k[:], w2_d[k])
                            yps = ppw.tile([P, CH], fp32, tag="w",
                                           name=f"y_ps_{j}_{k}")
                            for f in range(FT):
                                nc.tensor.matmul(yps[:], w2k[:, f, :], z[f][:],
                                                 start=(f == 0),
                                                 stop=(f == FT - 1))
                            nc.vector.scalar_tensor_tensor(
                                out=u1[k][:, sl], in0=yps[:],
                                scalar=b2_sb[:, k:k + 1], in1=u1[k][:, sl],
                                op0=OP.add, op1=OP.add)
                            stats_chunk(u1[k][:, sl], stp_bn2[j],
                                        k == 0, k == CT - 1)
                        bn2_io[j] = allreduce_chunk(u1pool, stp_bn2[j],
                                                    f"bn2_{j}")

            # ================= Phase 7: BN2 + output =================
            with tc.tile_pool(name="p7", bufs=1) as p7:
                bn2g_sb = p7.tile([1, T], fp32, name="bn2g_sb")
                nc.sync.dma_start(bn2g_sb[:], bn2g_d[:])
                bn2b_sb = p7.tile([1, T], fp32, name="bn2b_sb")
                nc.sync.dma_start(bn2b_sb[:], bn2b_d[:])
                for j in range(NCH):
                    sl = slice(j * CH, (j + 1) * CH)
                    sc, bi, _, _ = norm_params_chunk(
                        p7, bn2_io[j][1][:, 0:CH], bn2_io[j][1][:, CH:2 * CH],
                        NBC, f"bn2_{j}", g_row_sl=bn2g_sb[:, sl],
                        b_row_sl=bn2b_sb[:, sl])
                    for k in range(CT):
                        tmp = spool.tile([P, CH], fp32, tag="ntmp", bufs=2,
                                         name="ytmp")
                        nc.vector.tensor_mul(tmp[:], u1[k][:, sl], sc[:])
                        yk = spool.tile([P, CH], fp32, tag="yout", bufs=2,
                                        name=f"y_{k}_{j}")
                        nc.vector.tensor_add(yk[:], tmp[:], bi[:])
                        nc.sync.dma_start(yT_d[ts(k, P), sl], yk[:])

    nc.compile()
    return nc


def _get_program():
    global _PROG
    if _PROG is None:
        _PROG = _build()
    return _PROG


def _prep_shared(inputs):
    """Host-side weight folding + pre-tiling; identical for every core."""
    f32 = np.float32
    bf16 = ml_dtypes.bfloat16
    wq = np.asarray(inputs["wq"], f32)      # (H, C, D)
    wk = np.asarray(inputs["wk"], f32)
    wv = np.asarray(inputs["wv"], f32)
    wo = np.asarray(inputs["wo"], f32)      # (C, C)
    bo = np.asarray(inputs["bo"], f32)      # (C,)
    g1 = np.asarray(inputs["ln1_g"], f32)
    b1n = np.asarray(inputs["ln1_b"], f32)
    g2 = np.asarray(inputs["ln2_g"], f32)
    b2n = np.asarray(inputs["ln2_b"], f32)
    w1 = np.asarray(inputs["w1"], f32)      # (C, F)
    b1 = np.asarray(inputs["b1"], f32)      # (F,)
    w2 = np.asarray(inputs["w2"], f32)      # (F, C)
    b2 = np.asarray(inputs["b2"], f32)      # (C,)

    dscale = f32(D) ** f32(-0.5)
    # fold ln1 affine into qkv projections; q also takes 1/sqrt(D)
    wq2 = (wq * g1[None, :, None] * dscale).transpose(1, 0, 2).reshape(C, C)
    wk2 = (wk * g1[None, :, None]).transpose(1, 0, 2).reshape(C, C)
    wv2 = (wv * g1[None, :, None]).transpose(1, 0, 2).reshape(C, C)
    bq = (np.einsum("c,hcd->hd", b1n, wq) * dscale).reshape(C)
    bv = np.einsum("c,hcd->hd", b1n, wv).reshape(C)
    # k-side bias cancels in softmax (constant per row); v bias folds into bo
    bo2 = bo + bv @ wo
    w1f = g2[:, None] * w1
    b1f = b1 + b2n @ w1

    def lhst_tiles(w, n_out):
        # (C_in, n_out*P) -> (n_out, P, C_in//P, P):
        # [o, p, ki, n] = w[ki*P + p, o*P + n]
        ci = w.shape[0]
        return np.ascontiguousarray(
            w.reshape(ci // P, P, n_out, P).transpose(2, 1, 0, 3)
        ).astype(bf16)

    def cols(v, n):  # (n*P,) -> (P, n) with [p, i] = v[i*P + p]
        return np.ascontiguousarray(v.reshape(n, P).T, dtype=f32)

    def row(v):
        return np.ascontiguousarray(v.reshape(1, T), dtype=f32)

    return dict(
        wq=lhst_tiles(wq2, H), wk=lhst_tiles(wk2, H),
        wv=wv2.astype(bf16),
        bq=cols(bq, H), wo=lhst_tiles(wo, CT), bo=cols(bo2, CT),
        w1=lhst_tiles(w1f, FT), b1=cols(b1f, FT),
        w2=lhst_tiles(w2, CT), b2=cols(b2, CT),
        bn1g=row(np.asarray(inputs["bn1_g"], f32)),
        bn1b=row(np.asarray(inputs["bn1_b"], f32)),
        bn2g=row(np.asarray(inputs["bn2_g"], f32)),
        bn2b=row(np.asarray(inputs["bn2_b"], f32)),
    )


def _run(inputs, trace=False):
    from concourse import bass_utils
    nc = _get_program()
    x = np.asarray(inputs["x"], np.float32)
    shared = _prep_shared(inputs)
    in_maps = []
    for b in range(B):
        m = dict(shared)
        xt = np.ascontiguousarray(x[b].T)
        m["xT"] = xt
        m["xbf"] = xt.astype(ml_dtypes.bfloat16)
        in_maps.append(m)
    res = bass_utils.run_bass_kernel_spmd(
        nc, in_maps, core_ids=list(range(NCORES)), trace=trace)
    out = np.stack([res.results[b]["yT"].T for b in range(B)]).astype(np.float32)
    return out, res


def kernel(**inputs):
    out, _ = _run(inputs, trace=False)
    return out


# revision 17
# speedup vs baseline: 1.2132x; 1.0289x over previous
"""Trainium2 Bass kernel for nn_Block_50113678410401 (dense transformer block).

Strategy: data-parallel over the batch axis (B=8 -> 8 NeuronCores, one batch
element per core). All on-chip activations live in "layout A": feature axis on
SBUF partitions, token axis (T) on the free dimension, so no on-chip
transposes are needed (host pre-transposes x and post-transposes the output).

Per core:
  LN1 (stats via ones-matmul over partitions), per-head causal attention
  (no-max-sub exp softmax, denominator via ones-matmul, normalization via
  K=1 broadcast matmul + wide reciprocal), output projection + residual,
  BatchNorm over (B,C) with per-512-chunk cross-core AllReduces of
  (sum, sumsq) per T channel (the first chunk's collective overlaps the
  second chunk's matmuls), LN2, FFN (C -> 4C -> relu -> C), residual,
  second BatchNorm.

All big matmuls run in bf16 with fp32 PSUM accumulation; statistics,
softmax, residuals and normalizations are fp32. Weights arrive host-pretiled
so every weight DMA is contiguous per partition.

LayerNorm/projection affine parameters are folded into the weights on the
host: wq' = diag(ln1_g) wq / sqrt(D) (q also carries 1/sqrt(D)), k-side bias
drops out of softmax by shift invariance, v-side bias is folded into the
output-projection bias, ln2 affine is folded into w1/b1.
"""

import numpy as np
import ml_dtypes

B, T, C, H, D = 8, 1024, 1536, 12, 128
F = 4 * C            # 6144
P = 128
CT = C // P          # 12 c-tiles
FT = F // P          # 48 f-tiles
ST = T // P          # 8 s-tiles
CH = 512             # matmul free-dim chunk
NCH = T // CH        # 2 chunks
EPS = 1e-5
NCORES = 8
NBC = B * C          # BatchNorm count over (B, C)

_PROG = None


def _build():
    import concourse.bass as bass
    import concourse.mybir as mybir
    import concourse.tile as tile
    from concourse import bacc
    from concourse.masks import make_upper_triangular

    fp32 = mybir.dt.float32
    bf16 = mybir.dt.bfloat16
    AF = mybir.ActivationFunctionType
    OP = mybir.AluOpType
    ts = bass.ts

    nc = bacc.Bacc("TRN2", target_bir_lowering=False, debug=False,
                   enable_asserts=True, num_devices=NCORES)

    # ---- DRAM I/O (weights host-pretiled for contiguous DMA) ----
    xT_d = nc.dram_tensor("xT", (C, T), fp32, kind="ExternalInput").ap()
    xbf_d = nc.dram_tensor("xbf", (C, T), bf16, kind="ExternalInput").ap()
    wq_d = nc.dram_tensor("wq", (H, P, CT, P), bf16, kind="ExternalInput").ap()
    wk_d = nc.dram_tensor("wk", (H, P, CT, P), bf16, kind="ExternalInput").ap()
    wv_d = nc.dram_tensor("wv", (C, C), bf16, kind="ExternalInput").ap()
    bq_d = nc.dram_tensor("bq", (P, H), fp32, kind="ExternalInput").ap()
    wo_d = nc.dram_tensor("wo", (CT, P, H, P), bf16, kind="ExternalInput").ap()
    bo_d = nc.dram_tensor("bo", (P, CT), fp32, kind="ExternalInput").ap()
    w1_d = nc.dram_tensor("w1", (FT, P, CT, P), bf16, kind="ExternalInput").ap()
    b1_d = nc.dram_tensor("b1", (P, FT), fp32, kind="ExternalInput").ap()
    w2_d = nc.dram_tensor("w2", (CT, P, FT, P), bf16, kind="ExternalInput").ap()
    b2_d = nc.dram_tensor("b2", (P, CT), fp32, kind="ExternalInput").ap()
    bn1g_d = nc.dram_tensor("bn1g", (1, T), fp32, kind="ExternalInput").ap()
    bn1b_d = nc.dram_tensor("bn1b", (1, T), fp32, kind="ExternalInput").ap()
    bn2g_d = nc.dram_tensor("bn2g", (1, T), fp32, kind="ExternalInput").ap()
    bn2b_d = nc.dram_tensor("bn2b", (1, T), fp32, kind="ExternalInput").ap()
    yT_d = nc.dram_tensor("yT", (C, T), fp32, kind="ExternalOutput").ap()

    with tile.TileContext(nc) as tc:
        with tc.tile_pool(name="const", bufs=1) as cpool, \
             tc.tile_pool(name="scratch", bufs=1) as spool, \
             tc.tile_pool(name="u1p", bufs=1) as u1pool, \
             tc.tile_pool(name="ppw", bufs=6, space="PSUM") as ppw, \
             tc.tile_pool(name="pps", bufs=2, space="PSUM") as pps, \
             tc.tile_pool(name="dram", bufs=1, space="DRAM") as dpool:

            # ---- constants ----
            ones_bf = cpool.tile([P, 1], bf16, name="ones_bf")
            nc.vector.memset(ones_bf[:], 1.0)
            trimask = cpool.tile([P, P], bf16, name="trimask")
            make_upper_triangular(nc, trimask[:], val=1.0, diag=True)
            bq_sb = cpool.tile([P, H], fp32, name="bq_sb")
            nc.sync.dma_start(bq_sb[:], bq_d[:])
            bo_sb = cpool.tile([P, CT], fp32, name="bo_sb")
            nc.sync.dma_start(bo_sb[:], bo_d[:])
            b1_sb = cpool.tile([P, FT], fp32, name="b1_sb")
            nc.sync.dma_start(b1_sb[:], b1_d[:])
            b2_sb = cpool.tile([P, CT], fp32, name="b2_sb")
            nc.sync.dma_start(b2_sb[:], b2_d[:])

            # ---- helpers ----
            def bcast_into(dst_ap, row_ap, name, n=CH):
                """(1, n) fp32 SBUF row -> (P, n) via DRAM bounce; DMAs ride
                the vector queue (right after the row math that feeds them)
                so they never block weight loads or collectives."""
                dr = dpool.tile([1, n], fp32, tag="bcd", bufs=4,
                                name=f"{name}_dr")
                nc.scalar.dma_start(dr[:], row_ap)
                nc.scalar.dma_start(dst_ap, dr[:].to_broadcast((P, n)))

            ones1f = cpool.tile([1, P], fp32, name="ones1f")
            nc.vector.memset(ones1f[:], 1.0)

            def bc_mm_into(dst_ap, row_ap, name):
                """(1, CH) fp32 row -> (P, CH) SBUF via K=1 matmul + copy.
                Higher PE cost than bcast_into but ~3x lower latency; used on
                norm-param critical paths where the PE is idle anyway."""
                ps = ppw.tile([P, CH], fp32, tag="w", name=f"{name}_ps")
                nc.tensor.matmul(ps[:], ones1f[:], row_ap, start=True,
                                 stop=True)
                nc.scalar.copy(dst_ap, ps[:])

            # Packed stat psum tile: row 0 accumulates sum, row 32 sumsq.
            def stat_tiles(name):
                return [pps.tile([P, CH], fp32, tag="st", bufs=2,
                                 name=f"{name}_{j}") for j in range(NCH)]

            def stats_chunk(src_ap, stp_j, first, last, is_bf16=False):
                """Ones-matmul partial sums of src chunk ((P,CH)) and its
                square into packed stat rows."""
                if is_bf16:
                    cbf = src_ap
                else:
                    cbf_t = spool.tile([P, CH], bf16, tag="cast_bf", bufs=2,
                                       name="cbf")
                    nc.vector.tensor_copy(cbf_t[:], src_ap)
                    cbf = cbf_t[:]
                csq = spool.tile([P, CH], bf16, tag="cast_sq", bufs=2,
                                 name="csq")
                nc.scalar.square(csq[:], src_ap)
                nc.tensor.matmul(stp_j[0:1, :], ones_bf[:], cbf,
                                 start=first, stop=last)
                nc.tensor.matmul(stp_j[32:33, :], ones_bf[:], csq[:],
                                 start=first, stop=last)

            def allreduce_chunk(pool, stp_j, name):
                """AllReduce-add this chunk's packed (sum, sumsq) across
                cores. Returns the (1, 2*CH) global row."""
                loc = pool.tile([1, 2 * CH], fp32, tag="arloc", bufs=2,
                                name=f"{name}_loc")
                nc.scalar.copy(loc[:, 0:CH], stp_j[0:1, :])
                nc.scalar.copy(loc[:, CH:2 * CH], stp_j[32:33, :])
                cin = dpool.tile([1, 2 * CH], fp32, name=f"{name}_cin")
                cout = dpool.tile([1, 2 * CH], fp32, name=f"{name}_cout")
                nc.gpsimd.dma_start(cin[:], loc[:])
                nc.gpsimd.collective_compute(
                    "AllReduce", mybir.AluOpType.add,
                    replica_groups=[list(range(NCORES))],
                    ins=[cin.opt()], outs=[cout.opt()],
                )
                glob = pool.tile([1, 2 * CH], fp32, tag="arglob", bufs=2,
                                 name=f"{name}_glob")
                nc.gpsimd.dma_start(glob[:], cout[:])
                return loc, glob

            def norm_params_chunk(pool, s1_ap, s2_ap, count, name,
                                  g_row_sl=None, b_row_sl=None,
                                  sc_tag="nsc", bi_tag="nbi"):
                """Per-chunk normalization params, computed at row level
                (single-lane, cheap custom-DVE reciprocal), then broadcast
                to (P, CH) via DRAM-bounce DMA. Returns (sc_bc, bi_bc,
                scale_row, mean_row)."""
                m = pool.tile([1, CH], fp32, tag="rm", bufs=2,
                              name=f"{name}_m")
                nc.vector.tensor_scalar_mul(m[:], s1_ap, 1.0 / count)
                v = pool.tile([1, CH], fp32, tag="rv", bufs=2,
                              name=f"{name}_v")
                nc.vector.tensor_scalar_mul(v[:], s2_ap, 1.0 / count)
                msq = pool.tile([1, CH], fp32, tag="rq", bufs=2,
                                name=f"{name}_msq")
                nc.vector.tensor_mul(msq[:], m[:], m[:])
                nc.vector.tensor_sub(v[:], v[:], msq[:])
                nc.vector.tensor_scalar_add(v[:], v[:], EPS)
                nc.scalar.sqrt(v[:], v[:])
                scale = pool.tile([1, CH], fp32, tag="rs", bufs=2,
                                  name=f"{name}_srow")
                if g_row_sl is not None:
                    rc = pool.tile([1, CH], fp32, tag="rr", bufs=2,
                                   name=f"{name}_rc")
                    nc.vector.reciprocal_approx_fast(rc[:], v[:])
                    nc.vector.tensor_mul(scale[:], rc[:], g_row_sl)
                else:
                    nc.vector.reciprocal_approx_fast(scale[:], v[:])
                bias = pool.tile([1, CH], fp32, tag="rb", bufs=2,
                                 name=f"{name}_brow")
                nc.vector.tensor_mul(bias[:], m[:], scale[:])
                nc.vector.tensor_scalar_mul(bias[:], bias[:], -1.0)
                if b_row_sl is not None:
                    nc.vector.tensor_add(bias[:], bias[:], b_row_sl)
                sc_bc = pool.tile([P, CH], fp32, tag=sc_tag, bufs=2,
                                  name=f"{name}_scbc")
                bc_mm_into(sc_bc[:], scale[:], f"{name}_sc")
                bi_bc = pool.tile([P, CH], fp32, tag=bi_tag, bufs=2,
                                  name=f"{name}_bibc")
                bc_mm_into(bi_bc[:], bias[:], f"{name}_bi")
                return sc_bc, bi_bc, scale, m

            def affine_chunk(dst_ap, src_ap, sc_ap, bi_ap):
                """dst = src * sc + bi on one (P, CH) chunk."""
                tmp = spool.tile([P, CH], fp32, tag="ntmp", bufs=2,
                                 name="ntmp")
                nc.vector.tensor_mul(tmp[:], src_ap, sc_ap)
                nc.vector.tensor_add(dst_ap, tmp[:], bi_ap)

            u1 = []     # created at phase 4 (first use)
            o_nrm = []  # created at phase 3

            with tc.tile_pool(name="onrm", bufs=1) as opool:
                with tc.tile_pool(name="hT", bufs=1) as hpool:
                    hT = [hpool.tile([P, T], bf16, tag=f"h{k}", name=f"hT_{k}")
                          for k in range(CT)]
                    # ================= Phase 1: LN1 =================
                    with tc.tile_pool(name="p1", bufs=1) as p1:
                        stp = stat_tiles("ln1")
                        x_sb = []
                        for k in range(CT):
                            xk = p1.tile([P, T], bf16, tag=f"x{k}", name=f"x_{k}")
                            nc.sync.dma_start(xk[:], xbf_d[ts(k, P), :])
                            x_sb.append(xk)
                            for j in range(NCH):
                                sl = slice(j * CH, (j + 1) * CH)
                                stats_chunk(xk[:, sl], stp[j], k == 0,
                                            k == CT - 1, is_bf16=True)
                        for j in range(NCH):
                            sl = slice(j * CH, (j + 1) * CH)
                            sc, bi, _, _ = norm_params_chunk(
                                p1, stp[j][0:1, :], stp[j][32:33, :], C,
                                f"ln1_{j}")
                            for k in range(CT):
                                affine_chunk(hT[k][:, sl], x_sb[k][:, sl],
                                             sc[:], bi[:])

                    # ================= Phase 2: V for all heads ============
                    with tc.tile_pool(name="vall", bufs=1) as vpool:
                        Vall = [vpool.tile([P, C], bf16, tag=f"v{s}",
                                           name=f"V_{s}") for s in range(ST)]
                        with tc.tile_pool(name="wv", bufs=1) as wvpool:
                            wv_sb = []
                            for k in range(CT):
                                wvk = wvpool.tile([P, C], bf16, tag=f"wv{k}",
                                                  name=f"wv_{k}")
                                nc.sync.dma_start(wvk[:], wv_d[ts(k, P), :])
                                wv_sb.append(wvk)
                            for s in range(ST):
                                for n in range(C // CH):
                                    vps = ppw.tile([P, CH], fp32, tag="w",
                                                   name=f"v_ps_{s}_{n}")
                                    for k in range(CT):
                                        nc.tensor.matmul(
                                            vps[:], hT[k][:, ts(s, P)],
                                            wv_sb[k][:, ts(n, CH)],
                                            start=(k == 0), stop=(k == CT - 1))
                                    nc.scalar.copy(Vall[s][:, ts(n, CH)], vps[:])

                        # ============ Phase 3: per-head attention ==========
                        with tc.tile_pool(name="p3", bufs=1) as p3:
                            for h in range(H):
                                o_nrm.append(opool.tile(
                                    [P, T], bf16, tag=f"o{h}", name=f"on_{h}"))
                                wqh = p3.tile([P, CT, P], bf16, tag="wqh",
                                              bufs=2, name=f"wqh_{h}")
                                nc.sync.dma_start(wqh[:], wq_d[h])
                                wkh = p3.tile([P, CT, P], bf16, tag="wkh",
                                              bufs=2, name=f"wkh_{h}")
                                nc.sync.dma_start(wkh[:], wk_d[h])
                                qT = p3.tile([P, T], bf16, tag="qT", bufs=2,
                                             name=f"qT_{h}")
                                kT = p3.tile([P, T], bf16, tag="kT", bufs=2,
                                             name=f"kT_{h}")
                                for j in range(NCH):
                                    sl = slice(j * CH, (j + 1) * CH)
                                    qps = ppw.tile([P, CH], fp32, tag="w",
                                                   name=f"q_ps_{h}_{j}")
                                    for k in range(CT):
                                        nc.tensor.matmul(qps[:], wqh[:, k, :],
                                                         hT[k][:, sl],
                                                         start=(k == 0),
                                                         stop=(k == CT - 1))
                                    nc.scalar.activation(qT[:, sl], qps[:],
                                                         AF.Identity,
                                                         bias=bq_sb[:, h:h + 1],
                                                         scale=1.0)
                                    kps = ppw.tile([P, CH], fp32, tag="w",
                                                   name=f"k_ps_{h}_{j}")
                                    for k in range(CT):
                                        nc.tensor.matmul(kps[:], wkh[:, k, :],
                                                         hT[k][:, sl],
                                                         start=(k == 0),
                                                         stop=(k == CT - 1))
                                    nc.scalar.copy(kT[:, sl], kps[:])
                                # scores + exp (causal: s-tile covers t >= s*P)
                                aT = []
                                for s in range(ST):
                                    at = p3.tile([P, T], bf16, tag=f"a{s}",
                                                 bufs=1, name=f"aT_{h}_{s}")
                                    aT.append(at)
                                    for j in range(NCH):
                                        lo = max(j * CH, s * P)
                                        hi = (j + 1) * CH
                                        if lo >= hi:
                                            continue
                                        sps = ppw.tile([P, CH], fp32, tag="w",
                                                       name=f"s_ps_{h}_{s}_{j}")
                                        nc.tensor.matmul(sps[:, :hi - lo],
                                                         kT[:, ts(s, P)],
                                                         qT[:, lo:hi],
                                                         start=True, stop=True)
                                        nc.scalar.activation(at[:, lo:hi],
                                                             sps[:, :hi - lo],
                                                             AF.Exp)
                                    nc.vector.tensor_mul(at[:, ts(s, P)],
                                                         at[:, ts(s, P)],
                                                         trimask[:])
                                # denominators: packed psum, row 0 (j=0)
                                # and row 32 (j=1)
                                den_ps = pps.tile([P, CH], fp32, tag="st",
                                                  bufs=2, name=f"dn_{h}")
                                for j in range(NCH):
                                    r0 = 32 * j
                                    smax = min(ST, 4 * (j + 1))
                                    for s in range(smax):
                                        lo = max(0, s * P - j * CH)
                                        nc.tensor.matmul(
                                            den_ps[r0:r0 + 1, lo:CH],
                                            ones_bf[:],
                                            aT[s][:, j * CH + lo:(j + 1) * CH],
                                            start=(s == 0), stop=(s == smax - 1))
                                # r_bc = 1/den broadcast: copy row, K=1 mm,
                                # then wide reciprocal straight off PSUM
                                r_bc = p3.tile([P, T], fp32, tag="rbc", bufs=2,
                                               name=f"rbc_{h}")
                                for j in range(NCH):
                                    dj = p3.tile([1, CH], fp32, tag="den",
                                                 bufs=2, name=f"den_{h}_{j}")
                                    nc.scalar.copy(
                                        dj[:], den_ps[32 * j:32 * j + 1, :])
                                    rj = p3.tile([1, CH], fp32, tag="rrow",
                                                 bufs=2, name=f"rr_{h}_{j}")
                                    nc.vector.reciprocal_approx_fast(
                                        rj[:], dj[:])
                                    bcast_into(r_bc[:, j * CH:(j + 1) * CH],
                                               rj[:], f"rbc_{h}_{j}")
                                # attention @ V, then normalize
                                for j in range(NCH):
                                    smax = min(ST, 4 * (j + 1))
                                    ops_ = ppw.tile([P, CH], fp32, tag="w",
                                                    name=f"o_ps_{h}_{j}")
                                    for s in range(smax):
                                        lo = max(0, s * P - j * CH)
                                        nc.tensor.matmul(
                                            ops_[:, lo:CH],
                                            Vall[s][:, ts(h, P)],
                                            aT[s][:, j * CH + lo:(j + 1) * CH],
                                            start=(s == 0), stop=(s == smax - 1))
                                    sl = slice(j * CH, (j + 1) * CH)
                                    nc.vector.tensor_mul(o_nrm[h][:, sl],
                                                         ops_[:], r_bc[:, sl])

                # hT closed; Phase 4: out-proj + residual + BN1 stats
                # (j-outer so chunk 0's AllReduce overlaps chunk 1's matmuls)
                stp_bn1 = stat_tiles("bn1")
                bn1_io = [None, None]
                with tc.tile_pool(name="p4", bufs=1) as p4:
                    wok_sb = []
                    x2_sb = []
                    for k in range(CT):
                        wok = p4.tile([P, H, P], bf16, tag=f"wok{k}",
                                      name=f"wok_{k}")
                        nc.sync.dma_start(wok[:], wo_d[k])
                        wok_sb.append(wok)
                        x2k = p4.tile([P, T], fp32, tag=f"x2{k}",
                                      name=f"x2_{k}")
                        nc.sync.dma_start(x2k[:], xT_d[ts(k, P), :])
                        x2_sb.append(x2k)
                        u1.append(u1pool.tile([P, T], fp32, tag=f"u{k}",
                                              name=f"u1_{k}"))
                    for j in range(NCH):
                        sl = slice(j * CH, (j + 1) * CH)
                        for k in range(CT):
                            saps = ppw.tile([P, CH], fp32, tag="w",
                                            name=f"sa_ps_{k}_{j}")
                            for hh in range(H):
                                nc.tensor.matmul(saps[:], wok_sb[k][:, hh, :],
                                                 o_nrm[hh][:, sl],
                                                 start=(hh == 0),
                                                 stop=(hh == H - 1))
                            nc.vector.scalar_tensor_tensor(
                                out=u1[k][:, sl], in0=saps[:],
                                scalar=bo_sb[:, k:k + 1], in1=x2_sb[k][:, sl],
                                op0=OP.add, op1=OP.add)
                            stats_chunk(u1[k][:, sl], stp_bn1[j],
                                        k == 0, k == CT - 1)
                        bn1_io[j] = allreduce_chunk(u1pool, stp_bn1[j],
                                                    f"bn1_{j}")

            # ================= Phase 5: BN1 + LN2 =================
            with tc.tile_pool(name="h2T", bufs=1) as h2pool:
                h2T = [h2pool.tile([P, T], bf16, tag=f"h2{k}", name=f"h2_{k}")
                       for k in range(CT)]
                stp_bn2 = stat_tiles("bn2")
                bn2_io = [None, None]
                with tc.tile_pool(name="p5", bufs=1) as p5:
                    bn1g_sb = p5.tile([1, T], fp32, name="bn1g_sb")
                    nc.sync.dma_start(bn1g_sb[:], bn1g_d[:])
                    bn1b_sb = p5.tile([1, T], fp32, name="bn1b_sb")
                    nc.sync.dma_start(bn1b_sb[:], bn1b_d[:])
                    bn1_sc = []
                    bn1_bi = []
                    for j in range(NCH):
                        sl = slice(j * CH, (j + 1) * CH)
                        loc_j, glob_j = bn1_io[j]
                        sc, bi, s_row, _ = norm_params_chunk(
                            p5, glob_j[:, 0:CH], glob_j[:, CH:2 * CH],
                            NBC, f"bn1_{j}", g_row_sl=bn1g_sb[:, sl],
                            b_row_sl=bn1b_sb[:, sl], sc_tag="bnsc",
                            bi_tag="bnbi")
                        bn1_sc.append(sc)
                        bn1_bi.append(bi)
                        # LN2(BN1(u1)) == u1*A + B with A = s*rstd2,
                        # B = -mean_c(u1)*A, rstd2 = 1/sqrt(s^2*var_c(u1)+eps)
                        # -- derived from the LOCAL per-core stats rows, so no
                        # second stats pass is needed.
                        mc = p5.tile([1, CH], fp32, tag="rm", bufs=2,
                                     name=f"ln2m_{j}")
                        nc.vector.tensor_scalar_mul(mc[:], loc_j[:, 0:CH],
                                                    1.0 / C)
                        vc = p5.tile([1, CH], fp32, tag="rv", bufs=2,
                                     name=f"ln2v_{j}")
                        nc.vector.tensor_scalar_mul(vc[:], loc_j[:, CH:2 * CH],
                                                    1.0 / C)
                        msq = p5.tile([1, CH], fp32, tag="rq", bufs=2,
                                      name=f"ln2q_{j}")
                        nc.vector.tensor_mul(msq[:], mc[:], mc[:])
                        nc.vector.tensor_sub(vc[:], vc[:], msq[:])
                        s2r = p5.tile([1, CH], fp32, tag="rr", bufs=2,
                                      name=f"ln2s2_{j}")
                        nc.vector.tensor_mul(s2r[:], s_row[:], s_row[:])
                        nc.vector.tensor_mul(vc[:], vc[:], s2r[:])
                        nc.vector.tensor_scalar_add(vc[:], vc[:], EPS)
                        nc.scalar.sqrt(vc[:], vc[:])
                        r2 = p5.tile([1, CH], fp32, tag="rs", bufs=2,
                                     name=f"ln2r_{j}")
                        nc.vector.reciprocal_approx_fast(r2[:], vc[:])
                        arow = p5.tile([1, CH], fp32, tag="rb", bufs=2,
                                       name=f"ln2a_{j}")
                        nc.vector.tensor_mul(arow[:], r2[:], s_row[:])
                        brow = p5.tile([1, CH], fp32, tag="rb2", bufs=2,
                                       name=f"ln2b_{j}")
                        nc.vector.tensor_mul(brow[:], mc[:], arow[:])
                        nc.vector.tensor_scalar_mul(brow[:], brow[:], -1.0)
                        A = p5.tile([P, CH], fp32, tag="nsc", bufs=2,
                                    name=f"ln2A_{j}")
                        bc_mm_into(A[:], arow[:], f"ln2A_{j}")
                        Bt = p5.tile([P, CH], fp32, tag="nbi", bufs=2,
                                     name=f"ln2B_{j}")
                        bc_mm_into(Bt[:], brow[:], f"ln2B_{j}")
                        for k in range(CT):
                            affine_chunk(h2T[k][:, sl], u1[k][:, sl],
                                         A[:], Bt[:])
                    # deferred u1 -> BN1(u1) affines; these only gate the
                    # phase-6 residual adds, so they execute on DVE slack
                    # while the FFN matmuls run.
                    for j in range(NCH):
                        sl = slice(j * CH, (j + 1) * CH)
                        for k in range(CT):
                            affine_chunk(u1[k][:, sl], u1[k][:, sl],
                                         bn1_sc[j][:], bn1_bi[j][:])

                # ================= Phase 6: FFN =================
                with tc.tile_pool(name="p6", bufs=1) as p6:
                    for j in range(NCH):
                        sl = slice(j * CH, (j + 1) * CH)
                        z = []
                        for f in range(FT):
                            w1f = p6.tile([P, CT, P], bf16, tag="w1f", bufs=2,
                                          name=f"w1f_{j}_{f}")
                            nc.sync.dma_start(w1f[:], w1_d[f])
                            zps = ppw.tile([P, CH], fp32, tag="w",
                                           name=f"z_ps_{j}_{f}")
                            for k in range(CT):
                                nc.tensor.matmul(zps[:], w1f[:, k, :],
                                                 h2T[k][:, sl],
                                                 start=(k == 0),
                                                 stop=(k == CT - 1))
                            zf = p6.tile([P, CH], bf16, tag=f"z{f}",
                                         name=f"z_{j}_{f}")
                            nc.scalar.activation(zf[:], zps[:], AF.Relu,
                                                 bias=b1_sb[:, f:f + 1],
                                                 scale=1.0)
                            z.append(zf)
                        for k in range(CT):
                            w2k = p6.tile([P, FT, P], bf16, tag="w2k", bufs=2,
                                          name=f"w2k_{j}_{k}")
                            nc.sync.dma_start(w2k[:], w2_d[k])
                            yps = ppw.tile([P, CH], fp32, tag="w",
                                           name=f"y_ps_{j}_{k}")
                            for f in range(FT):
                                nc.tensor.matmul(yps[:], w2k[:, f, :], z[f][:],
                                                 start=(f == 0),
                                                 stop=(f == FT - 1))
                            nc.vector.scalar_tensor_tensor(
                                out=u1[k][:, sl], in0=yps[:],
                                scalar=b2_sb[:, k:k + 1], in1=u1[k][:, sl],
                                op0=OP.add, op1=OP.add)
                            stats_chunk(u1[k][:, sl], stp_bn2[j],
                                        k == 0, k == CT - 1)
                        bn2_io[j] = allreduce_chunk(u1pool, stp_bn2[j],
                                                    f"bn2_{j}")

            # ================= Phase 7: BN2 + output =================
            with tc.tile_pool(name="p7", bufs=1) as p7:
                bn2g_sb = p7.tile([1, T], fp32, name="bn2g_sb")
                nc.sync.dma_start(bn2g_sb[:], bn2g_d[:])
                bn2b_sb = p7.tile([1, T], fp32, name="bn2b_sb")
                nc.sync.dma_start(bn2b_sb[:], bn2b_d[:])
                for j in range(NCH):
                    sl = slice(j * CH, (j + 1) * CH)
                    sc, bi, _, _ = norm_params_chunk(
                        p7, bn2_io[j][1][:, 0:CH], bn2_io[j][1][:, CH:2 * CH],
                        NBC, f"bn2_{j}", g_row_sl=bn2g_sb[:, sl],
                        b_row_sl=bn2b_sb[:, sl])
                    for k in range(CT):
                        tmp = spool.tile([P, CH], fp32, tag="ntmp", bufs=2,
                                         name="ytmp")
                        nc.vector.tensor_mul(tmp[:], u1[k][:, sl], sc[:])
                        yk = spool.tile([P, CH], fp32, tag="yout", bufs=2,
                                        name=f"y_{k}_{j}")
                        nc.vector.tensor_add(yk[:], tmp[:], bi[:])
                        nc.sync.dma_start(yT_d[ts(k, P), sl], yk[:])

    nc.compile()
    return nc


def _get_program():
    global _PROG
    if _PROG is None:
        _PROG = _build()
    return _PROG


def _prep_shared(inputs):
    """Host-side weight folding + pre-tiling; identical for every core."""
    f32 = np.float32
    bf16 = ml_dtypes.bfloat16
    wq = np.asarray(inputs["wq"], f32)      # (H, C, D)
    wk = np.asarray(inputs["wk"], f32)
    wv = np.asarray(inputs["wv"], f32)
    wo = np.asarray(inputs["wo"], f32)      # (C, C)
    bo = np.asarray(inputs["bo"], f32)      # (C,)
    g1 = np.asarray(inputs["ln1_g"], f32)
    b1n = np.asarray(inputs["ln1_b"], f32)
    g2 = np.asarray(inputs["ln2_g"], f32)
    b2n = np.asarray(inputs["ln2_b"], f32)
    w1 = np.asarray(inputs["w1"], f32)      # (C, F)
    b1 = np.asarray(inputs["b1"], f32)      # (F,)
    w2 = np.asarray(inputs["w2"], f32)      # (F, C)
    b2 = np.asarray(inputs["b2"], f32)      # (C,)

    dscale = f32(D) ** f32(-0.5)
    # fold ln1 affine into qkv projections; q also takes 1/sqrt(D)
    wq2 = (wq * g1[None, :, None] * dscale).transpose(1, 0, 2).reshape(C, C)
    wk2 = (wk * g1[None, :, None]).transpose(1, 0, 2).reshape(C, C)
    wv2 = (wv * g1[None, :, None]).transpose(1, 0, 2).reshape(C, C)
    bq = (np.einsum("c,hcd->hd", b1n, wq) * dscale).reshape(C)
    bv = np.einsum("c,hcd->hd", b1n, wv).reshape(C)
    # k-side bias cancels in softmax (constant per row); v bias folds into bo
    bo2 = bo + bv @ wo
    w1f = g2[:, None] * w1
    b1f = b1 + b2n @ w1

    def lhst_tiles(w, n_out):
        # (C_in, n_out*P) -> (n_out, P, C_in//P, P):
        # [o, p, ki, n] = w[ki*P + p, o*P + n]
        ci = w.shape[0]
        return np.ascontiguousarray(
            w.reshape(ci // P, P, n_out, P).transpose(2, 1, 0, 3)
        ).astype(bf16)

    def cols(v, n):  # (n*P,) -> (P, n) with [p, i] = v[i*P + p]
        return np.ascontiguousarray(v.reshape(n, P).T, dtype=f32)

    def row(v):
        return np.ascontiguousarray(v.reshape(1, T), dtype=f32)

    return dict(
        wq=lhst_tiles(wq2, H), wk=lhst_tiles(wk2, H),
        wv=wv2.astype(bf16),
        bq=cols(bq, H), wo=lhst_tiles(wo, CT), bo=cols(bo2, CT),
        w1=lhst_tiles(w1f, FT), b1=cols(b1f, FT),
        w2=lhst_tiles(w2, CT), b2=cols(b2, CT),
        bn1g=row(np.asarray(inputs["bn1_g"], f32)),
        bn1b=row(np.asarray(inputs["bn1_b"], f32)),
        bn2g=row(np.asarray(inputs["bn2_g"], f32)),
        bn2b=row(np.asarray(inputs["bn2_b"], f32)),
    )


def _run(inputs, trace=False):
    from concourse import bass_utils
    nc = _get_program()
    x = np.asarray(inputs["x"], np.float32)
    shared = _prep_shared(inputs)
    in_maps = []
    for b in range(B):
        m = dict(shared)
        xt = np.ascontiguousarray(x[b].T)
        m["xT"] = xt
        m["xbf"] = xt.astype(ml_dtypes.bfloat16)
        in_maps.append(m)
    res = bass_utils.run_bass_kernel_spmd(
        nc, in_maps, core_ids=list(range(NCORES)), trace=trace)
    out = np.stack([res.results[b]["yT"].T for b in range(B)]).astype(np.float32)
    return out, res


def kernel(**inputs):
    out, _ = _run(inputs, trace=False)
    return out
